# revision 30
# baseline (speedup 1.0000x reference)
"""DreamAttention (GQA + RoPE + causal) on 8 trn2 NeuronCores.

Sharding: DP=2 over batch x sequence-parallel over q-tiles. Core c ->
(batch b = c // 4, seq rank r = c % 4). Core r owns q-tiles
[r, 7-r, 8+r, 15-r] (128 rows each, ascending) — every core gets exactly 34
k-tile-blocks of causal attention work, so the load is perfectly balanced.

K/V projection is seq-sharded: each core computes K^T/V for ONLY its own
512 contiguous positions (1/4 of S), then an AllGather over the 4-core
batch group assembles the full K/V. The collective runs on TOPSP+SDMA
silicon; it is sandwiched between the two halves of the Q projection so
its SDMA traffic overlaps PE work whose weights are already buffered
(wqp ring depth 8). This removes the 4x redundant K/V compute (~150us of
PE time) a collective-free version would pay.

All matmul operands are bf16 (fp32 PSUM accumulation). The first ~230us
is simultaneously PE- and DMA-bound (~51MB of weights/activations at
~240GB/s effective), so DMA queue ORDER is tuned: sync carries A0's
weights then Wo (ring-throttled into phase C), scalar carries xt-half +
xq + the wq ring, gpsimd carries the collective + gather-back. Host-side
layouts give every big stream >=7KB contiguous per-partition lines.

Per-core dataflow:
  - A1 heads 0-13: Q projection + fused Q-RoPE -> qt[:, h, :].
  - A0: K/V projection for own 512 positions (K-RoPE fused), V
    PE-transposed to natural layout; chunks to internal DRAM; AllGather
    [[0-3],[4-7]]; gather back into kt_t [128, 4kv, S] / vn.
  - A1 heads 14-27 (collective in flight underneath).
  - B: attention per (head, seg): k-tiles are packed into 6 two-bank
    [128,1024] PSUM segments so exp runs as 6 wide ACTIVATEs instead of
    16 narrow ones (the ACT engine costs ~200ns/op + w/1.2GHz; this cut
    exp from 7.5 to ~5.5us/head). Matmul outputs never cross a 2KB PSUM
    bank boundary (the 384-wide pairs sit at offsets 0/512). The additive
    causal mask for the first live 128-block of each k-tile rides the PE
    as a += maskT^T @ I accumulate — keeping it on-PE keeps the PE dense
    enough that HAM stays at K=8/8 (off-PE masking measurably dropped the
    PE clock). P^T -> exp -> PV with a one-segment software-pipeline lag;
    the softmax denominator (ones-matmul partition reduce + PE
    outer-product broadcast + normalize) for head h is emitted INSIDE
    head h+1's segment loop so the PE FIFO never stalls on the DVE chain.
  - C: o_proj (full Wo); attnT stationary, Wo moving, accumulate over 28
    head-chunks; output rows are core-owned -> DMA straight out as bf16.
Host reassembles the 8 cores' row-slices into the full [2, 2048, 3584] output.
"""

import math

import numpy as np
from ml_dtypes import bfloat16

import concourse.bass as bass
import concourse.mybir as mybir
import concourse.tile as tile
from concourse import bacc
from concourse.bass_utils import run_bass_kernel_spmd
from concourse.masks import make_identity

F32 = mybir.dt.float32
BF16 = mybir.dt.bfloat16

B, S, D = 2, 2048, 3584
H, KVH, HD = 28, 4, 128
ROPE_THETA = 1000000.0
GQ = H // KVH   # 7 q heads per kv head
DKT = D // 128  # 28 k-tiles over D
SC = 512        # per-core owned K/V chunk (S / 4)
NKT = S // 128  # 16 k tiles over sequence
NST = SC // 128  # 4 seq tiles per owned chunk
NDC = 7         # output D chunks of 512
NQT = 4         # q-tiles owned per core
QW = NQT * 128  # 512 q columns per core
SCALE = 1.0 / math.sqrt(HD)
PVDEPTH = 3     # attention software-pipeline depth (S runs ahead of PV)
RG = [[0, 1, 2, 3], [4, 5, 6, 7]]  # batch groups for the K/V AllGather


def _qtiles(r):
    """Ascending q-tile ids owned by seq-rank r; sum of (t+1) == 34 for all r."""
    return [r, 7 - r, 8 + r, 15 - r]


def _wof(kti):
    # Live-suffix width for k-tile kti. Rank-independent: every rank's
    # ascending tile list [t0<t1<t2<t3] satisfies t0<=3, 4<=t1<=7, 8<=t2<=11,
    # 12<=t3<=15, so #(tiles >= kti) == 4 - kti//4 for all ranks.
    return 128 * (4 - kti // 4)


_NC_CACHE = {}


def _build_nc():
    key = "nc"
    if key in _NC_CACHE:
        return _NC_CACHE[key]

    nc = bacc.Bacc("TRN2", target_bir_lowering=False, debug=False, num_devices=8)

    xq_d = nc.dram_tensor("xq", [4, 128, DKT // 4, QW], BF16, kind="ExternalInput").ap()
    xt_d = nc.dram_tensor("xt", [4, 128, DKT // 4, SC], BF16, kind="ExternalInput").ap()
    wq_d = nc.dram_tensor("wq", [H, 128, DKT, 128], BF16, kind="ExternalInput").ap()
    wkv_d = nc.dram_tensor(
        "wkv", [2 * KVH, 2, 128, DKT // 2, 128], BF16, kind="ExternalInput"
    ).ap()
    wo_d = nc.dram_tensor(
        "wo", [NDC, 4, 128, DKT // 4, 512], BF16, kind="ExternalInput"
    ).ap()
    cosq_d = nc.dram_tensor("cosq", [128, QW], BF16, kind="ExternalInput").ap()
    sinq_d = nc.dram_tensor("sinq", [128, QW], BF16, kind="ExternalInput").ap()
    cosk_d = nc.dram_tensor("cosk", [128, SC], BF16, kind="ExternalInput").ap()
    sink_d = nc.dram_tensor("sink", [128, SC], BF16, kind="ExternalInput").ap()
    mask_d = nc.dram_tensor("mask", [NKT, 128, 128], BF16, kind="ExternalInput").ap()
    out_d = nc.dram_tensor("out", [NQT, 128, D], BF16, kind="ExternalOutput").ap()

    # K/V AllGather bounce buffers (collectives can't touch kernel I/O).
    # cc_in[0] = own K^T chunk [128, 4kv, 512pos]; cc_in[1] = own V natural
    # chunk [128pos-in-tile, 4kv, 4tile, 128d]. AllGather concatenates rank
    # shards along the leading axis of cc_out.
    cc_in = nc.dram_tensor("cc_in", [2, 128, 2048], BF16)
    # Shared addr_space needs >4-core groups; Local works for 4-core AG.
    cc_out = nc.dram_tensor("cc_out", [4, 2, 128, 2048], BF16)

    with tile.TileContext(nc) as tc:
        with tc.tile_pool(name="persist", bufs=1) as persist:
            # qt doubles as the attention-output buffer: att(h) overwrites
            # qt[:, h, :] once head h's scores are done.
            qt = persist.tile([128, H, QW], BF16, name="qt")
            kt_t = persist.tile([128, KVH, S], BF16, name="kt")
            vn = persist.tile([128, KVH, NKT, 128], BF16, name="vn")
            ident = persist.tile([128, 128], BF16, name="ident")
            ones = persist.tile([128, 1], BF16, name="ones")
            ones_row = persist.tile([1, 128], BF16, name="ones_row")

            make_identity(nc, ident)
            nc.vector.memset(ones, 1.0)
            nc.vector.memset(ones_row, 1.0)

            def rope(dst, cos_ap, sin_ap, width, tmp, eng):
                # eng: DMA queue for the rotate-half copies. Phase A1 must
                # NOT use gpsimd — its engine stream blocks in the
                # collective's wait_ge and would stall A1's DVE chain.
                t = tmp[:, :width]
                eng.dma_start(out=t[0:64, :], in_=dst[64:128, :])
                eng.dma_start(out=t[64:128, :], in_=dst[0:64, :])
                nc.vector.tensor_mul(t, t, sin_ap)
                nc.vector.tensor_mul(dst, dst, cos_ap)
                nc.vector.tensor_add(dst, dst, t)

            # ---- Phase A0: K/V projection for OWN 512 positions, K-RoPE
            # fused; then AllGather across the 4-core batch group.
            # ---- Phase A1: Q projection + fused Q-RoPE (AllGather hides
            # under this).
            # One pool scope for both phases: separate scopes would make
            # A1's xq/wq prefetch wait for A0's SBUF to release (measured
            # ~38us of PE stall at the seam).
            with (
                tc.tile_pool(name="ropetab", bufs=1) as ropetab,
                tc.tile_pool(name="ropep", bufs=2) as ropep,
                tc.tile_pool(name="xtp", bufs=4) as xtp,
                tc.tile_pool(name="wkvp", bufs=6) as wkvp,
                tc.tile_pool(name="kvchunk", bufs=1) as kvchunk,
                tc.tile_pool(name="xqp", bufs=1) as xqp,
                tc.tile_pool(name="wqp", bufs=7) as wqp,
                tc.tile_pool(name="qtab", bufs=1) as qtab,
                tc.tile_pool(name="qrtmp", bufs=2) as qrtmp,
                tc.tile_pool(name="ps_kv", bufs=2, space="PSUM") as ps_kv,
                tc.tile_pool(name="ps_tr", bufs=2, space="PSUM") as ps_tr,
                tc.tile_pool(name="ps_a", bufs=3, space="PSUM") as ps_a,
            ):
                # DMA issue order tuned so the first matmul starts ~3us in:
                # wkv[0,0] leads the sync queue, xt[0] right behind it.
                wkv_tiles = {}

                def load_wkv(ct):
                    # all K/V weights on sync: the scalar queue belongs to
                    # the xq/wq stream, whose pool ring-waits must never
                    # sit AHEAD of A0's weight refills.
                    for hf in range(2):
                        wblk = wkvp.tile([128, DKT // 2, 128], BF16, name="wkv")
                        nc.sync.dma_start(out=wblk, in_=wkv_d[ct, hf])
                        wkv_tiles[(ct, hf)] = wblk

                cosk = ropetab.tile([128, SC], BF16, name="cosk")
                sink = ropetab.tile([128, SC], BF16, name="sink")
                nc.scalar.dma_start(out=cosk, in_=cosk_d)
                nc.scalar.dma_start(out=sink, in_=sink_d)
                load_wkv(0)
                xts = []
                for ch in range(4):
                    xtile = xtp.tile([128, DKT // 4, SC], BF16, name="xt")
                    eng = nc.sync if ch < 2 else nc.scalar
                    eng.dma_start(out=xtile, in_=xt_d[ch])
                    for j in range(DKT // 4):
                        xts.append(xtile[:, j, :])
                for ct in range(1, 3):
                    load_wkv(ct)
                # A1 prefetch: xq + first wq blocks stream on the SCALAR
                # queue behind A0's xt half, so they fill during A0's
                # compute without ever delaying A0's sync-side weights.
                cosq = qtab.tile([128, QW], BF16, name="cosq")
                sinq = qtab.tile([128, QW], BF16, name="sinq")
                nc.scalar.dma_start(out=cosq, in_=cosq_d)
                nc.scalar.dma_start(out=sinq, in_=sinq_d)
                xq = xqp.tile([128, DKT, QW], BF16, name="xq")
                for ch in range(4):
                    nc.scalar.dma_start(
                        out=xq[:, ch * (DKT // 4) : (ch + 1) * (DKT // 4), :],
                        in_=xq_d[ch],
                    )
                wq_tiles = {}

                def load_wq(ct):
                    wblk = wqp.tile([128, DKT, 128], BF16, name="wq")
                    nc.scalar.dma_start(out=wblk, in_=wq_d[ct])
                    wq_tiles[ct] = wblk

                for ct in range(7):
                    load_wq(ct)

                def a1_heads(h_lo, h_hi):
                    # Q projection + fused Q-RoPE for heads [h_lo, h_hi)
                    for ct in range(h_lo, h_hi):
                        if ct + 7 < H:
                            load_wq(ct + 7)
                        wblk = wq_tiles.pop(ct)
                        psum = ps_a.tile([128, QW], F32, name="pp")
                        for kti in range(DKT):
                            nc.tensor.matmul(
                                psum,
                                wblk[:, kti, :],
                                xq[:, kti, :],
                                start=(kti == 0),
                                stop=(kti == DKT - 1),
                            )
                        nc.vector.tensor_copy(qt[:, ct, :], psum)
                        tmp = qrtmp.tile([128, QW], BF16, name="qrtmp")
                        rope(qt[:, ct, :], cosq, sinq, QW, tmp, nc.scalar)

                # A1 front half FIRST: the uncontended early DMA window
                # feeds xq + the wq ring; A0 + the AllGather run in the
                # middle so the collective's SDMA traffic overlaps A1's
                # back half, whose weights are already buffered (wqp=6).
                a1_heads(0, 14)

                kt_own = kvchunk.tile([128, KVH, SC], BF16, name="kt_own")
                vt_own = kvchunk.tile([128, KVH, SC], BF16, name="vt_own")
                vn_own = kvchunk.tile([128, KVH, NST, 128], BF16, name="vn_own")
                for ct in range(2 * KVH):  # 0-3: K heads, 4-7: V
                    if ct + 3 < 2 * KVH:
                        load_wkv(ct + 3)
                    psum = ps_kv.tile([128, SC], F32, name="pp")
                    for hf in range(2):
                        wblk = wkv_tiles.pop((ct, hf))
                        for kti in range(DKT // 2):
                            gkt = hf * (DKT // 2) + kti
                            nc.tensor.matmul(
                                psum,
                                wblk[:, kti, :],
                                xts[gkt],
                                start=(gkt == 0),
                                stop=(gkt == DKT - 1),
                            )
                    if ct < KVH:
                        nc.vector.tensor_copy(kt_own[:, ct, :], psum)
                        tmp = ropep.tile([128, SC], BF16, name="ropetmp")
                        rope(kt_own[:, ct, :], cosk, sink, SC, tmp, nc.gpsimd)
                    else:
                        nc.vector.tensor_copy(vt_own[:, ct - KVH, :], psum)
                # V^T -> V natural (4 s-tiles x 4 heads)
                for kv in range(KVH):
                    for sti in range(NST):
                        ptr = ps_tr.tile([128, 128], BF16, name="ptr")
                        nc.tensor.transpose(
                            ptr,
                            vt_own[:, kv, sti * 128 : (sti + 1) * 128],
                            ident,
                        )
                        nc.vector.tensor_copy(vn_own[:, kv, sti, :], ptr)
                # own chunks -> DRAM bounce, AllGather, gather back
                nc.gpsimd.dma_start(
                    out=cc_in[0].rearrange("p (kv s) -> p kv s", kv=KVH),
                    in_=kt_own,
                )
                nc.gpsimd.dma_start(
                    out=cc_in[1].rearrange(
                        "p (kv st d) -> p kv st d", kv=KVH, st=NST
                    ),
                    in_=vn_own,
                )
                nc.gpsimd.collective_compute(
                    "AllGather",
                    mybir.AluOpType.bypass,
                    replica_groups=RG,
                    ins=[cc_in.ap()],
                    outs=[cc_out.ap()],
                )
                # gather-back on gpsimd: it already blocks in the
                # collective's wait_ge, and using sync here would
                # head-of-line-block phase A1's weight loads behind the
                # collective (measured: ~50us PE stall).
                for rr in range(4):
                    nc.gpsimd.dma_start(
                        out=kt_t[:, :, rr * SC : (rr + 1) * SC],
                        in_=cc_out[rr, 0].rearrange("p (kv s) -> p kv s", kv=KVH),
                    )
                    nc.gpsimd.dma_start(
                        out=vn[:, :, rr * NST : (rr + 1) * NST, :],
                        in_=cc_out[rr, 1].rearrange(
                            "p (kv st d) -> p kv st d", kv=KVH, st=NST
                        ),
                    )

                # ---- A1 back half (collective in flight underneath) ----
                a1_heads(14, H)

            # ---- Phase B+C share the wop pool so Wo prefetches during B ----
            with (
                tc.tile_pool(name="wop", bufs=8) as wop,
                tc.tile_pool(name="outp", bufs=3) as outp,
            ):
                wo_tiles = {}

                def load_wo(dc):
                    # 4 ring-throttled chunk DMAs per dc: the ring keeps the
                    # bulk of Wo streaming during phase C (not during the
                    # collective window, which a single early DMA would hit).
                    tl = []
                    for ch in range(4):
                        wt = wop.tile([128, DKT // 4, 512], BF16, name="wo")
                        nc.sync.dma_start(out=wt, in_=wo_d[dc, ch])
                        tl.append(wt)
                    wo_tiles[dc] = tl

                load_wo(0)
                load_wo(1)

                # ---- Phase B: attention, 28 heads on the core's 512 q ----
                # k-tiles are packed into 6 PSUM "segments" per head (each a
                # 2-bank [128, 1024] f32 tile) so exp runs as 6 wide
                # ACTIVATEs instead of 16 narrow ones — the scalar engine's
                # ~200ns/op overhead made exp the phase bottleneck (7.5us
                # -> ~5.5us per head). The 0/1 causal-mask multiplies run on
                # GpSimd (otherwise idle); denominator accumulation stays on
                # DVE.  SEGS: list of ([(kti, column offset)...], exp width).
                # A matmul output must NOT cross a 2KB PSUM bank boundary,
                # so the 384-wide pairs sit at offsets 0/512 and exp spans
                # the (unread) garbage hole in between.
                SEGS = [
                    ([(0, 0), (1, 512)], 1024),
                    ([(2, 0), (3, 512)], 1024),
                    ([(4, 0), (5, 512)], 896),
                    ([(6, 0), (7, 512)], 896),
                    ([(8, 0), (9, 256), (10, 512), (11, 768)], 1024),
                    ([(12, 0), (13, 128), (14, 256), (15, 384)], 512),
                ]
                with (
                    tc.tile_pool(name="ptp", bufs=3) as ptp,
                    tc.tile_pool(name="accp", bufs=2) as accp,
                    tc.tile_pool(name="smallp", bufs=4) as smallp,
                    tc.tile_pool(name="maskp", bufs=1) as maskp,
                    tc.tile_pool(name="ps_s", bufs=2, space="PSUM") as ps_s,
                    tc.tile_pool(name="ps_o", bufs=2, space="PSUM") as ps_o,
                    tc.tile_pool(name="ps_r", bufs=1, space="PSUM") as ps_r,
                ):
                    # mask via gpsimd: the scalar queue is busy with the wq
                    # ring until late A1, and sync with wo prefetch.
                    mask_t = maskp.tile([128, NKT, 128], BF16, name="mask_t")
                    nc.gpsimd.dma_start(
                        out=mask_t, in_=mask_d.rearrange("k p q -> p k q")
                    )

                    # deferred softmax-denominator pipeline: emit head h-1's
                    # partition-reduce / reciprocal / broadcast / normalize
                    # interleaved into head h's segment loop so the PE FIFO
                    # never stalls on the DVE accumulation chain.
                    pend_norm = []

                    def norm_step(step):
                        if not pend_norm:
                            return
                        if step == 0:
                            ph, pacc, ppo = pend_norm[0]
                            # denominator: single partition-reduce matmul
                            psum_r = ps_r.tile([1, QW], F32, name="psr")
                            nc.tensor.matmul(
                                psum_r, ones, pacc, start=True, stop=True
                            )
                            rec = smallp.tile([1, QW], F32, name="rec")
                            nc.vector.reciprocal_approx_fast(rec, psum_r)
                            rec_bf = smallp.tile([1, QW], BF16, name="rec_bf")
                            nc.vector.tensor_copy(rec_bf, rec)
                            pend_norm[0] = (ph, pacc, ppo, rec_bf)
                        elif step == 1:
                            ph, pacc, ppo, rec_bf = pend_norm[0]
                            # broadcast along partitions via PE outer product
                            bcast = ps_r.tile([128, QW], F32, name="bcast")
                            nc.tensor.matmul(
                                bcast, ones_row, rec_bf, start=True, stop=True
                            )
                            bcast_sb = smallp.tile([128, QW], F32, name="bcast_sb")
                            nc.vector.tensor_copy(bcast_sb, bcast)
                            pend_norm[0] = (ph, pacc, ppo, bcast_sb)
                        else:
                            ph, pacc, ppo, bcast_sb = pend_norm.pop(0)
                            # fused normalize + PSUM->SBUF, overwriting head
                            # ph's spent q columns
                            nc.vector.tensor_mul(qt[:, ph, :], ppo, bcast_sb)

                    for h in range(H):
                        kv = h // GQ
                        psum_o = ps_o.tile([128, QW], F32, name="po")
                        acc = accp.tile([128, QW], BF16, name="acc")
                        pend = []

                        def emit_pv(ent, psum_o=psum_o, kv=kv):
                            kti, lo, w, pt, off = ent
                            nc.tensor.matmul(
                                psum_o[:, lo : lo + w],
                                vn[:, kv, kti, :],
                                pt[:, off : off + w],
                                start=(kti == 0),
                                stop=(kti == NKT - 1),
                            )

                        for si, (ktis, segw) in enumerate(SEGS):
                            w = _wof(ktis[0][0])
                            lo = QW - w
                            pseg = ps_s.tile([128, 1024], F32, name="pss")
                            for kti, off in ktis:
                                nc.tensor.matmul(
                                    pseg[:, off : off + w],
                                    kt_t[:, kv, kti * 128 : (kti + 1) * 128],
                                    qt[:, h, lo:],
                                    start=True,
                                    stop=False,
                                )
                                # additive causal mask folded into the score
                                # accumulation group on the PE itself: the
                                # first live 128-block (the only one that can
                                # be diagonal or dead) gets += maskT^T @ I.
                                # Cheapest engine per op (~107ns, LDW-bound),
                                # and the extra PE density keeps HAM at
                                # K=8/8 — off-PE masking measurably dropped
                                # the PE clock to ~1.4GHz in phase B.
                                nc.tensor.matmul(
                                    pseg[:, off : off + 128],
                                    mask_t[:, kti, :],
                                    ident,
                                    start=False,
                                    stop=True,
                                )
                            pt = ptp.tile([128, 1024], BF16, name="pt")
                            nc.scalar.activation(
                                pt[:, :segw],
                                pseg[:, :segw],
                                mybir.ActivationFunctionType.Exp,
                            )
                            for kti, off in ktis:
                                # running softmax-denominator partial on DVE
                                if kti == 0:
                                    nc.vector.tensor_copy(
                                        acc, pt[:, off : off + w]
                                    )
                                else:
                                    nc.vector.tensor_add(
                                        acc[:, lo:],
                                        acc[:, lo:],
                                        pt[:, off : off + w],
                                    )
                                pend.append((kti, lo, w, pt, off))
                            # one-segment lag: PVs of segment si-1 emit here
                            while len(pend) > len(ktis):
                                emit_pv(pend.pop(0))
                            if si == 1:
                                norm_step(0)
                            elif si == 2:
                                norm_step(1)
                            elif si == 3:
                                norm_step(2)
                        while pend:
                            emit_pv(pend.pop(0))
                        pend_norm.append((h, acc, psum_o))
                    # drain the last head's normalization
                    norm_step(0)
                    norm_step(1)
                    norm_step(2)

                # ---- Phase C: o_proj (full Wo); rows are core-owned ----
                with tc.tile_pool(name="ps_c", bufs=2, space="PSUM") as ps_c:
                    for dc in range(NDC):
                        if dc + 2 < NDC:
                            load_wo(dc + 2)
                        tl = wo_tiles.pop(dc)
                        for q in range(NQT):
                            psum = ps_c.tile([128, 512], F32, name="pp")
                            for ct in range(DKT):
                                nc.tensor.matmul(
                                    psum,
                                    qt[:, ct, q * 128 : (q + 1) * 128],
                                    tl[ct // 7][:, ct % 7, :],
                                    start=(ct == 0),
                                    stop=(ct == DKT - 1),
                                )
                            ob = outp.tile([128, 512], BF16, name="ob")
                            nc.vector.tensor_copy(ob, psum)
                            nc.scalar.dma_start(
                                out=out_d[q, :, dc * 512 : (dc + 1) * 512],
                                in_=ob,
                            )

    nc.finalize()
    _NC_CACHE[key] = nc
    return nc


def _host_inputs(hidden_states, Wq, Wk, Wv, Wo):
    hidden = np.asarray(hidden_states, dtype=np.float32)
    Wq = np.asarray(Wq, dtype=np.float32) * np.float32(SCALE)
    Wk = np.asarray(Wk, dtype=np.float32)
    Wv = np.asarray(Wv, dtype=np.float32)
    Wo = np.asarray(Wo, dtype=np.float32)

    inv_freq = 1.0 / ROPE_THETA ** (np.arange(0, HD, 2, dtype=np.float32) / HD)
    t = np.arange(S, dtype=np.float32)
    freqs = np.outer(t, inv_freq)  # [S, 64]
    cos_t = np.cos(freqs.T)  # [64, S]
    sin_t = np.sin(freqs.T)
    cosk = np.concatenate([cos_t, cos_t], axis=0).astype(bfloat16)  # [128, S]
    sink = np.concatenate([-sin_t, sin_t], axis=0).astype(bfloat16)

    # shared weight layouts (identical for every core)
    wq = np.ascontiguousarray(
        Wq.reshape(DKT, 128, H, 128).transpose(2, 1, 0, 3)
    ).astype(bfloat16)  # [h, p, kt, c]
    wk4 = Wk.reshape(DKT, 128, KVH, 128)
    wv4 = Wv.reshape(DKT, 128, KVH, 128)
    wkv = np.empty((2 * KVH, 2, 128, DKT // 2, 128), np.float32)
    for ct in range(KVH):
        for hf in range(2):
            ktsl = slice(hf * (DKT // 2), (hf + 1) * (DKT // 2))
            wkv[ct, hf] = wk4[ktsl, :, ct, :].transpose(1, 0, 2)
            wkv[KVH + ct, hf] = wv4[ktsl, :, ct, :].transpose(1, 0, 2)
    wkv = wkv.astype(bfloat16)
    wo = np.ascontiguousarray(
        Wo.reshape(4, DKT // 4, 128, NDC, 512).transpose(3, 0, 2, 1, 4)
    ).astype(bfloat16)  # [dc, ch, p, kt, d]

    in_maps = []
    for core in range(8):
        b, r = core // 4, core % 4
        tiles = _qtiles(r)
        qpos = np.concatenate(
            [np.arange(t0 * 128, (t0 + 1) * 128) for t0 in tiles]
        )  # [512] ascending global q positions
        xq = np.ascontiguousarray(
            hidden[b][qpos].reshape(QW, 4, DKT // 4, 128).transpose(1, 3, 2, 0)
        ).astype(bfloat16)  # [ch, p, kt, q]
        # own contiguous K/V chunk: positions [r*SC, (r+1)*SC)
        xt = np.ascontiguousarray(
            hidden[b][r * SC : (r + 1) * SC]
            .reshape(SC, 4, DKT // 4, 128)
            .transpose(1, 3, 2, 0)
        ).astype(bfloat16)  # [ch, p, kt, s]
        cosq = np.ascontiguousarray(cosk[:, qpos])
        sinq = np.ascontiguousarray(sink[:, qpos])
        cosk_own = np.ascontiguousarray(cosk[:, r * SC : (r + 1) * SC])
        sink_own = np.ascontiguousarray(sink[:, r * SC : (r + 1) * SC])
        # mask[kt]: [128, 128] TRANSPOSED additive mask ([q, k] layout — it is
        # the stationary operand of a += maskT^T @ I accumulate on the PE) for
        # the FIRST live block of the suffix (columns QW-w .. QW-w+128).
        # Triangular when that block's q-tile equals kt (the diagonal),
        # all -inf when the block is non-causal (dead), all-zero otherwise.
        mask = np.zeros((NKT, 128, 128), np.float32)
        for kti in range(NKT):
            lo = QW - _wof(kti)
            kk = kti * 128 + np.arange(128)[None, :]
            qq = qpos[lo : lo + 128, None]
            mask[kti] = np.where(kk <= qq, 0.0, -30000.0)
        mask = mask.astype(bfloat16)
        in_maps.append(
            {
                "xq": xq,
                "xt": xt,
                "wq": wq,
                "wkv": wkv,
                "wo": wo,
                "cosq": cosq,
                "sinq": sinq,
                "cosk": cosk_own,
                "sink": sink_own,
                "mask": mask,
            }
        )
    return in_maps


def kernel(hidden_states, Wq, Wk, Wv, Wo, trace=False):
    nc = _build_nc()
    in_maps = _host_inputs(hidden_states, Wq, Wk, Wv, Wo)
    res = run_bass_kernel_spmd(nc, in_maps, list(range(8)), trace=trace)
    out = np.empty((B, S, D), dtype=np.float32)
    for core in range(8):
        b, r = core // 4, core % 4
        o = np.asarray(res.results[core]["out"], dtype=np.float32)
        for j, t0 in enumerate(_qtiles(r)):
            out[b, t0 * 128 : (t0 + 1) * 128, :] = o[j]
    if trace:
        kernel.last_exec_time_ns = res.exec_time_ns
    return out


# revision 31
# speedup vs baseline: 1.0150x; 1.0150x over previous
"""DreamAttention (GQA + RoPE + causal) on 8 trn2 NeuronCores.

Sharding: DP=2 over batch x sequence-parallel over q-tiles. Core c ->
(batch b = c // 4, seq rank r = c % 4). Core r owns q-tiles
[r, 7-r, 8+r, 15-r] (128 rows each, ascending) — every core gets exactly 34
k-tile-blocks of causal attention work, so the load is perfectly balanced.

K/V projection is seq-sharded: each core computes K^T/V for ONLY its own
512 contiguous positions (1/4 of S), then an AllGather over the 4-core
batch group assembles the full K/V. The collective runs on TOPSP+SDMA
silicon; it is sandwiched between the two halves of the Q projection so
its SDMA traffic overlaps PE work whose weights are already buffered
(wqp ring depth 6). This removes the 4x redundant K/V compute (~150us of
PE time) a collective-free version would pay.

All matmul operands are bf16 (fp32 PSUM accumulation). The first ~230us
is simultaneously PE- and DMA-bound (~51MB of weights/activations at
~240GB/s effective), so DMA queue ORDER is tuned: sync carries A0's
weights then Wo (ring-throttled into phase C), scalar carries xt-half +
xq + the wq ring, gpsimd carries the collective + gather-back. Host-side
layouts give every big stream >=7KB contiguous per-partition lines.

Per-core dataflow:
  - A1 heads 0-13: Q projection + fused Q-RoPE -> qt[:, h, :].
  - A0: K/V projection for own 512 positions (K-RoPE fused), V
    PE-transposed to natural layout; chunks to internal DRAM; AllGather
    [[0-3],[4-7]]; gather back into kt_t [128, 4kv, S] / vn.
  - A1 heads 14-27 (collective in flight underneath).
  - B: attention per (head, seg): k-tiles are packed into 6 two-bank
    [128,1024] PSUM segments so exp runs as 6 wide ACTIVATEs instead of
    16 narrow ones (the ACT engine costs ~200ns/op + w/1.2GHz; this cut
    exp from 7.5 to ~5.5us/head). Matmul outputs never cross a 2KB PSUM
    bank boundary (the 384-wide pairs sit at offsets 0/512). The additive
    causal mask for the first live 128-block of each k-tile rides the PE
    as a += maskT^T @ I accumulate — keeping it on-PE keeps the PE dense
    enough that HAM stays at K=8/8 (off-PE masking measurably dropped the
    PE clock). P^T -> exp -> PV with a one-segment software-pipeline lag;
    the softmax denominator (ones-matmul partition reduce + PE
    outer-product broadcast + normalize) for head h is emitted INSIDE
    head h+1's segment loop so the PE FIFO never stalls on the DVE chain.
  - C: o_proj (full Wo); attnT stationary, Wo moving, accumulate over 28
    head-chunks; output rows are core-owned -> DMA straight out as bf16.
Host reassembles the 8 cores' row-slices into the full [2, 2048, 3584] output.
"""

import math

import numpy as np
from ml_dtypes import bfloat16

import concourse.bass as bass
import concourse.mybir as mybir
import concourse.tile as tile
from concourse import bacc
from concourse.bass_utils import run_bass_kernel_spmd
from concourse.masks import make_identity

F32 = mybir.dt.float32
BF16 = mybir.dt.bfloat16

B, S, D = 2, 2048, 3584
H, KVH, HD = 28, 4, 128
ROPE_THETA = 1000000.0
GQ = H // KVH   # 7 q heads per kv head
DKT = D // 128  # 28 k-tiles over D
SC = 512        # per-core owned K/V chunk (S / 4)
NKT = S // 128  # 16 k tiles over sequence
NST = SC // 128  # 4 seq tiles per owned chunk
NDC = 7         # output D chunks of 512
NQT = 4         # q-tiles owned per core
QW = NQT * 128  # 512 q columns per core
SCALE = 1.0 / math.sqrt(HD)
PVDEPTH = 3     # attention software-pipeline depth (S runs ahead of PV)
RG = [[0, 1, 2, 3], [4, 5, 6, 7]]  # batch groups for the K/V AllGather


def _qtiles(r):
    """Ascending q-tile ids owned by seq-rank r; sum of (t+1) == 34 for all r."""
    return [r, 7 - r, 8 + r, 15 - r]


def _wof(kti):
    # Live-suffix width for k-tile kti. Rank-independent: every rank's
    # ascending tile list [t0<t1<t2<t3] satisfies t0<=3, 4<=t1<=7, 8<=t2<=11,
    # 12<=t3<=15, so #(tiles >= kti) == 4 - kti//4 for all ranks.
    return 128 * (4 - kti // 4)


_NC_CACHE = {}


def _build_nc():
    key = "nc"
    if key in _NC_CACHE:
        return _NC_CACHE[key]

    nc = bacc.Bacc("TRN2", target_bir_lowering=False, debug=False, num_devices=8)

    xq_d = nc.dram_tensor("xq", [4, 128, DKT // 4, QW], BF16, kind="ExternalInput").ap()
    xt_d = nc.dram_tensor("xt", [4, 128, DKT // 4, SC], BF16, kind="ExternalInput").ap()
    wq_d = nc.dram_tensor("wq", [H, 128, DKT, 128], BF16, kind="ExternalInput").ap()
    wkv_d = nc.dram_tensor(
        "wkv", [2 * KVH, 2, 128, DKT // 2, 128], BF16, kind="ExternalInput"
    ).ap()
    wo_d = nc.dram_tensor(
        "wo", [NDC, 4, 128, DKT // 4, 512], BF16, kind="ExternalInput"
    ).ap()
    cosq_d = nc.dram_tensor("cosq", [128, QW], BF16, kind="ExternalInput").ap()
    sinq_d = nc.dram_tensor("sinq", [128, QW], BF16, kind="ExternalInput").ap()
    cosk_d = nc.dram_tensor("cosk", [128, SC], BF16, kind="ExternalInput").ap()
    sink_d = nc.dram_tensor("sink", [128, SC], BF16, kind="ExternalInput").ap()
    mask_d = nc.dram_tensor("mask", [NKT, 128, 128], BF16, kind="ExternalInput").ap()
    out_d = nc.dram_tensor("out", [NQT, 128, D], BF16, kind="ExternalOutput").ap()

    # K/V AllGather bounce buffers (collectives can't touch kernel I/O).
    # cc_in[0] = own K^T chunk [128, 4kv, 512pos]; cc_in[1] = own V natural
    # chunk [128pos-in-tile, 4kv, 4tile, 128d]. AllGather concatenates rank
    # shards along the leading axis of cc_out.
    cc_in = nc.dram_tensor("cc_in", [2, 128, 2048], BF16)
    # Shared addr_space needs >4-core groups; Local works for 4-core AG.
    cc_out = nc.dram_tensor("cc_out", [4, 2, 128, 2048], BF16)

    with tile.TileContext(nc) as tc:
        with tc.tile_pool(name="persist", bufs=1) as persist:
            # qt doubles as the attention-output buffer: att(h) overwrites
            # qt[:, h, :] once head h's scores are done.
            qt = persist.tile([128, H, QW], BF16, name="qt")
            kt_t = persist.tile([128, KVH, S], BF16, name="kt")
            vn = persist.tile([128, KVH, NKT, 128], BF16, name="vn")
            ident = persist.tile([128, 128], BF16, name="ident")
            ones = persist.tile([128, 1], BF16, name="ones")
            ones_row = persist.tile([1, 128], BF16, name="ones_row")

            make_identity(nc, ident)
            nc.vector.memset(ones, 1.0)
            nc.vector.memset(ones_row, 1.0)

            def rope(dst, cos_ap, sin_ap, width, tmp, eng):
                # eng: DMA queue for the rotate-half copies. Phase A1 must
                # NOT use gpsimd — its engine stream blocks in the
                # collective's wait_ge and would stall A1's DVE chain.
                t = tmp[:, :width]
                eng.dma_start(out=t[0:64, :], in_=dst[64:128, :])
                eng.dma_start(out=t[64:128, :], in_=dst[0:64, :])
                nc.vector.tensor_mul(t, t, sin_ap)
                nc.vector.tensor_mul(dst, dst, cos_ap)
                nc.vector.tensor_add(dst, dst, t)

            # ---- Phase A0: K/V projection for OWN 512 positions, K-RoPE
            # fused; then AllGather across the 4-core batch group.
            # ---- Phase A1: Q projection + fused Q-RoPE (AllGather hides
            # under this).
            # One pool scope for both phases: separate scopes would make
            # A1's xq/wq prefetch wait for A0's SBUF to release (measured
            # ~38us of PE stall at the seam).
            with (
                tc.tile_pool(name="ropetab", bufs=1) as ropetab,
                tc.tile_pool(name="ropep", bufs=2) as ropep,
                tc.tile_pool(name="xtp", bufs=4) as xtp,
                tc.tile_pool(name="wkvp", bufs=6) as wkvp,
                tc.tile_pool(name="kvchunk", bufs=1) as kvchunk,
                tc.tile_pool(name="xqp", bufs=1) as xqp,
                tc.tile_pool(name="wqp", bufs=6) as wqp,
                tc.tile_pool(name="qtab", bufs=1) as qtab,
                tc.tile_pool(name="qrtmp", bufs=2) as qrtmp,
                tc.tile_pool(name="ps_kv", bufs=2, space="PSUM") as ps_kv,
                tc.tile_pool(name="ps_tr", bufs=2, space="PSUM") as ps_tr,
                tc.tile_pool(name="ps_a", bufs=3, space="PSUM") as ps_a,
            ):
                # DMA issue order tuned so the first matmul starts ~3us in:
                # wkv[0,0] leads the sync queue, xt[0] right behind it.
                wkv_tiles = {}

                def load_wkv(ct):
                    # all K/V weights on sync: the scalar queue belongs to
                    # the xq/wq stream, whose pool ring-waits must never
                    # sit AHEAD of A0's weight refills.
                    for hf in range(2):
                        wblk = wkvp.tile([128, DKT // 2, 128], BF16, name="wkv")
                        nc.sync.dma_start(out=wblk, in_=wkv_d[ct, hf])
                        wkv_tiles[(ct, hf)] = wblk

                cosk = ropetab.tile([128, SC], BF16, name="cosk")
                sink = ropetab.tile([128, SC], BF16, name="sink")
                nc.scalar.dma_start(out=cosk, in_=cosk_d)
                nc.scalar.dma_start(out=sink, in_=sink_d)
                load_wkv(0)
                xts = []
                for ch in range(4):
                    xtile = xtp.tile([128, DKT // 4, SC], BF16, name="xt")
                    eng = nc.sync if ch < 2 else nc.scalar
                    eng.dma_start(out=xtile, in_=xt_d[ch])
                    for j in range(DKT // 4):
                        xts.append(xtile[:, j, :])
                for ct in range(1, 3):
                    load_wkv(ct)
                # A1 prefetch: xq + first wq blocks stream on the SCALAR
                # queue behind A0's xt half, so they fill during A0's
                # compute without ever delaying A0's sync-side weights.
                cosq = qtab.tile([128, QW], BF16, name="cosq")
                sinq = qtab.tile([128, QW], BF16, name="sinq")
                nc.scalar.dma_start(out=cosq, in_=cosq_d)
                nc.scalar.dma_start(out=sinq, in_=sinq_d)
                xq = xqp.tile([128, DKT, QW], BF16, name="xq")
                for ch in range(4):
                    nc.scalar.dma_start(
                        out=xq[:, ch * (DKT // 4) : (ch + 1) * (DKT // 4), :],
                        in_=xq_d[ch],
                    )
                wq_tiles = {}

                def load_wq(ct):
                    wblk = wqp.tile([128, DKT, 128], BF16, name="wq")
                    nc.scalar.dma_start(out=wblk, in_=wq_d[ct])
                    wq_tiles[ct] = wblk

                for ct in range(6):
                    load_wq(ct)

                def a1_heads(h_lo, h_hi):
                    # Q projection + fused Q-RoPE for heads [h_lo, h_hi)
                    for ct in range(h_lo, h_hi):
                        if ct + 6 < H:
                            load_wq(ct + 6)
                        wblk = wq_tiles.pop(ct)
                        psum = ps_a.tile([128, QW], F32, name="pp")
                        for kti in range(DKT):
                            nc.tensor.matmul(
                                psum,
                                wblk[:, kti, :],
                                xq[:, kti, :],
                                start=(kti == 0),
                                stop=(kti == DKT - 1),
                            )
                        nc.vector.tensor_copy(qt[:, ct, :], psum)
                        tmp = qrtmp.tile([128, QW], BF16, name="qrtmp")
                        rope(qt[:, ct, :], cosq, sinq, QW, tmp, nc.scalar)

                # A1 front half FIRST: the uncontended early DMA window
                # feeds xq + the wq ring; A0 + the AllGather run in the
                # middle so the collective's SDMA traffic overlaps A1's
                # back half, whose weights are already buffered (wqp=6).
                a1_heads(0, 14)

                kt_own = kvchunk.tile([128, KVH, SC], BF16, name="kt_own")
                vt_own = kvchunk.tile([128, KVH, SC], BF16, name="vt_own")
                vn_own = kvchunk.tile([128, KVH, NST, 128], BF16, name="vn_own")
                for ct in range(2 * KVH):  # 0-3: K heads, 4-7: V
                    if ct + 3 < 2 * KVH:
                        load_wkv(ct + 3)
                    psum = ps_kv.tile([128, SC], F32, name="pp")
                    for hf in range(2):
                        wblk = wkv_tiles.pop((ct, hf))
                        for kti in range(DKT // 2):
                            gkt = hf * (DKT // 2) + kti
                            nc.tensor.matmul(
                                psum,
                                wblk[:, kti, :],
                                xts[gkt],
                                start=(gkt == 0),
                                stop=(gkt == DKT - 1),
                            )
                    if ct < KVH:
                        nc.vector.tensor_copy(kt_own[:, ct, :], psum)
                        tmp = ropep.tile([128, SC], BF16, name="ropetmp")
                        rope(kt_own[:, ct, :], cosk, sink, SC, tmp, nc.gpsimd)
                    else:
                        nc.vector.tensor_copy(vt_own[:, ct - KVH, :], psum)
                # V^T -> V natural (4 s-tiles x 4 heads)
                for kv in range(KVH):
                    for sti in range(NST):
                        ptr = ps_tr.tile([128, 128], BF16, name="ptr")
                        nc.tensor.transpose(
                            ptr,
                            vt_own[:, kv, sti * 128 : (sti + 1) * 128],
                            ident,
                        )
                        nc.vector.tensor_copy(vn_own[:, kv, sti, :], ptr)
                # own chunks -> DRAM bounce, AllGather, gather back
                nc.gpsimd.dma_start(
                    out=cc_in[0].rearrange("p (kv s) -> p kv s", kv=KVH),
                    in_=kt_own,
                )
                nc.gpsimd.dma_start(
                    out=cc_in[1].rearrange(
                        "p (kv st d) -> p kv st d", kv=KVH, st=NST
                    ),
                    in_=vn_own,
                )
                nc.gpsimd.collective_compute(
                    "AllGather",
                    mybir.AluOpType.bypass,
                    replica_groups=RG,
                    ins=[cc_in.ap()],
                    outs=[cc_out.ap()],
                )
                # gather-back on gpsimd: it already blocks in the
                # collective's wait_ge, and using sync here would
                # head-of-line-block phase A1's weight loads behind the
                # collective (measured: ~50us PE stall).
                for rr in range(4):
                    nc.gpsimd.dma_start(
                        out=kt_t[:, :, rr * SC : (rr + 1) * SC],
                        in_=cc_out[rr, 0].rearrange("p (kv s) -> p kv s", kv=KVH),
                    )
                    nc.gpsimd.dma_start(
                        out=vn[:, :, rr * NST : (rr + 1) * NST, :],
                        in_=cc_out[rr, 1].rearrange(
                            "p (kv st d) -> p kv st d", kv=KVH, st=NST
                        ),
                    )

                # ---- A1 back half (collective in flight underneath) ----
                a1_heads(14, H)

            # ---- Phase B+C share the wop pool so Wo prefetches during B ----
            with (
                tc.tile_pool(name="wop", bufs=8) as wop,
                tc.tile_pool(name="outp", bufs=3) as outp,
            ):
                wo_tiles = {}

                def load_wo(dc):
                    # 4 ring-throttled chunk DMAs per dc: the ring keeps the
                    # bulk of Wo streaming during phase C (not during the
                    # collective window, which a single early DMA would hit).
                    tl = []
                    for ch in range(4):
                        wt = wop.tile([128, DKT // 4, 512], BF16, name="wo")
                        nc.sync.dma_start(out=wt, in_=wo_d[dc, ch])
                        tl.append(wt)
                    wo_tiles[dc] = tl

                load_wo(0)
                load_wo(1)

                # ---- Phase B: attention, 28 heads on the core's 512 q ----
                # k-tiles are packed into 6 PSUM "segments" per head (each a
                # 2-bank [128, 1024] f32 tile) so exp runs as 6 wide
                # ACTIVATEs instead of 16 narrow ones — the scalar engine's
                # ~200ns/op overhead made exp the phase bottleneck (7.5us
                # -> ~5.5us per head). The 0/1 causal-mask multiplies run on
                # GpSimd (otherwise idle); denominator accumulation stays on
                # DVE.  SEGS: list of ([(kti, column offset)...], exp width).
                # A matmul output must NOT cross a 2KB PSUM bank boundary,
                # so the 384-wide pairs sit at offsets 0/512 and exp spans
                # the (unread) garbage hole in between.
                SEGS = [
                    ([(0, 0), (1, 512)], 1024),
                    ([(2, 0), (3, 512)], 1024),
                    ([(4, 0), (5, 512)], 896),
                    ([(6, 0), (7, 512)], 896),
                    ([(8, 0), (9, 256), (10, 512), (11, 768)], 1024),
                    ([(12, 0), (13, 128), (14, 256), (15, 384)], 512),
                ]
                with (
                    tc.tile_pool(name="ptp", bufs=3) as ptp,
                    tc.tile_pool(name="accp", bufs=2) as accp,
                    tc.tile_pool(name="smallp", bufs=4) as smallp,
                    tc.tile_pool(name="maskp", bufs=1) as maskp,
                    tc.tile_pool(name="ps_s", bufs=2, space="PSUM") as ps_s,
                    tc.tile_pool(name="ps_o", bufs=2, space="PSUM") as ps_o,
                    tc.tile_pool(name="ps_r", bufs=1, space="PSUM") as ps_r,
                ):
                    # mask via gpsimd: the scalar queue is busy with the wq
                    # ring until late A1, and sync with wo prefetch.
                    mask_t = maskp.tile([128, NKT, 128], BF16, name="mask_t")
                    nc.gpsimd.dma_start(
                        out=mask_t, in_=mask_d.rearrange("k p q -> p k q")
                    )

                    # deferred softmax-denominator pipeline: emit head h-1's
                    # partition-reduce / reciprocal / broadcast / normalize
                    # interleaved into head h's segment loop so the PE FIFO
                    # never stalls on the DVE accumulation chain.
                    pend_norm = []

                    def norm_step(step):
                        if not pend_norm:
                            return
                        if step == 0:
                            ph, pacc, ppo = pend_norm[0]
                            # denominator: single partition-reduce matmul
                            psum_r = ps_r.tile([1, QW], F32, name="psr")
                            nc.tensor.matmul(
                                psum_r, ones, pacc, start=True, stop=True
                            )
                            rec = smallp.tile([1, QW], F32, name="rec")
                            nc.vector.reciprocal_approx_fast(rec, psum_r)
                            rec_bf = smallp.tile([1, QW], BF16, name="rec_bf")
                            nc.vector.tensor_copy(rec_bf, rec)
                            pend_norm[0] = (ph, pacc, ppo, rec_bf)
                        elif step == 1:
                            ph, pacc, ppo, rec_bf = pend_norm[0]
                            # broadcast along partitions via PE outer product
                            bcast = ps_r.tile([128, QW], F32, name="bcast")
                            nc.tensor.matmul(
                                bcast, ones_row, rec_bf, start=True, stop=True
                            )
                            bcast_sb = smallp.tile([128, QW], F32, name="bcast_sb")
                            nc.vector.tensor_copy(bcast_sb, bcast)
                            pend_norm[0] = (ph, pacc, ppo, bcast_sb)
                        else:
                            ph, pacc, ppo, bcast_sb = pend_norm.pop(0)
                            # fused normalize + PSUM->SBUF, overwriting head
                            # ph's spent q columns
                            nc.vector.tensor_mul(qt[:, ph, :], ppo, bcast_sb)

                    for h in range(H):
                        kv = h // GQ
                        psum_o = ps_o.tile([128, QW], F32, name="po")
                        acc = accp.tile([128, QW], BF16, name="acc")
                        pend = []

                        def emit_pv(ent, psum_o=psum_o, kv=kv):
                            kti, lo, w, pt, off = ent
                            nc.tensor.matmul(
                                psum_o[:, lo : lo + w],
                                vn[:, kv, kti, :],
                                pt[:, off : off + w],
                                start=(kti == 0),
                                stop=(kti == NKT - 1),
                            )

                        for si, (ktis, segw) in enumerate(SEGS):
                            w = _wof(ktis[0][0])
                            lo = QW - w
                            pseg = ps_s.tile([128, 1024], F32, name="pss")
                            for kti, off in ktis:
                                nc.tensor.matmul(
                                    pseg[:, off : off + w],
                                    kt_t[:, kv, kti * 128 : (kti + 1) * 128],
                                    qt[:, h, lo:],
                                    start=True,
                                    stop=False,
                                )
                                # additive causal mask folded into the score
                                # accumulation group on the PE itself: the
                                # first live 128-block (the only one that can
                                # be diagonal or dead) gets += maskT^T @ I.
                                # Cheapest engine per op (~107ns, LDW-bound),
                                # and the extra PE density keeps HAM at
                                # K=8/8 — off-PE masking measurably dropped
                                # the PE clock to ~1.4GHz in phase B.
                                nc.tensor.matmul(
                                    pseg[:, off : off + 128],
                                    mask_t[:, kti, :],
                                    ident,
                                    start=False,
                                    stop=True,
                                )
                            pt = ptp.tile([128, 1024], BF16, name="pt")
                            nc.scalar.activation(
                                pt[:, :segw],
                                pseg[:, :segw],
                                mybir.ActivationFunctionType.Exp,
                            )
                            for kti, off in ktis:
                                # running softmax-denominator partial on DVE
                                if kti == 0:
                                    nc.vector.tensor_copy(
                                        acc, pt[:, off : off + w]
                                    )
                                else:
                                    nc.vector.tensor_add(
                                        acc[:, lo:],
                                        acc[:, lo:],
                                        pt[:, off : off + w],
                                    )
                                pend.append((kti, lo, w, pt, off))
                            # one-segment lag: PVs of segment si-1 emit here
                            while len(pend) > len(ktis):
                                emit_pv(pend.pop(0))
                            if si == 1:
                                norm_step(0)
                            elif si == 2:
                                norm_step(1)
                            elif si == 3:
                                norm_step(2)
                        while pend:
                            emit_pv(pend.pop(0))
                        pend_norm.append((h, acc, psum_o))
                    # drain the last head's normalization
                    norm_step(0)
                    norm_step(1)
                    norm_step(2)

                # ---- Phase C: o_proj (full Wo); rows are core-owned ----
                with tc.tile_pool(name="ps_c", bufs=2, space="PSUM") as ps_c:
                    for dc in range(NDC):
                        if dc + 2 < NDC:
                            load_wo(dc + 2)
                        tl = wo_tiles.pop(dc)
                        for q in range(NQT):
                            psum = ps_c.tile([128, 512], F32, name="pp")
                            for ct in range(DKT):
                                nc.tensor.matmul(
                                    psum,
                                    qt[:, ct, q * 128 : (q + 1) * 128],
                                    tl[ct // 7][:, ct % 7, :],
                                    start=(ct == 0),
                                    stop=(ct == DKT - 1),
                                )
                            ob = outp.tile([128, 512], BF16, name="ob")
                            nc.vector.tensor_copy(ob, psum)
                            nc.scalar.dma_start(
                                out=out_d[q, :, dc * 512 : (dc + 1) * 512],
                                in_=ob,
                            )

    nc.finalize()
    _NC_CACHE[key] = nc
    return nc


def _host_inputs(hidden_states, Wq, Wk, Wv, Wo):
    hidden = np.asarray(hidden_states, dtype=np.float32)
    Wq = np.asarray(Wq, dtype=np.float32) * np.float32(SCALE)
    Wk = np.asarray(Wk, dtype=np.float32)
    Wv = np.asarray(Wv, dtype=np.float32)
    Wo = np.asarray(Wo, dtype=np.float32)

    inv_freq = 1.0 / ROPE_THETA ** (np.arange(0, HD, 2, dtype=np.float32) / HD)
    t = np.arange(S, dtype=np.float32)
    freqs = np.outer(t, inv_freq)  # [S, 64]
    cos_t = np.cos(freqs.T)  # [64, S]
    sin_t = np.sin(freqs.T)
    cosk = np.concatenate([cos_t, cos_t], axis=0).astype(bfloat16)  # [128, S]
    sink = np.concatenate([-sin_t, sin_t], axis=0).astype(bfloat16)

    # shared weight layouts (identical for every core)
    wq = np.ascontiguousarray(
        Wq.reshape(DKT, 128, H, 128).transpose(2, 1, 0, 3)
    ).astype(bfloat16)  # [h, p, kt, c]
    wk4 = Wk.reshape(DKT, 128, KVH, 128)
    wv4 = Wv.reshape(DKT, 128, KVH, 128)
    wkv = np.empty((2 * KVH, 2, 128, DKT // 2, 128), np.float32)
    for ct in range(KVH):
        for hf in range(2):
            ktsl = slice(hf * (DKT // 2), (hf + 1) * (DKT // 2))
            wkv[ct, hf] = wk4[ktsl, :, ct, :].transpose(1, 0, 2)
            wkv[KVH + ct, hf] = wv4[ktsl, :, ct, :].transpose(1, 0, 2)
    wkv = wkv.astype(bfloat16)
    wo = np.ascontiguousarray(
        Wo.reshape(4, DKT // 4, 128, NDC, 512).transpose(3, 0, 2, 1, 4)
    ).astype(bfloat16)  # [dc, ch, p, kt, d]

    in_maps = []
    for core in range(8):
        b, r = core // 4, core % 4
        tiles = _qtiles(r)
        qpos = np.concatenate(
            [np.arange(t0 * 128, (t0 + 1) * 128) for t0 in tiles]
        )  # [512] ascending global q positions
        xq = np.ascontiguousarray(
            hidden[b][qpos].reshape(QW, 4, DKT // 4, 128).transpose(1, 3, 2, 0)
        ).astype(bfloat16)  # [ch, p, kt, q]
        # own contiguous K/V chunk: positions [r*SC, (r+1)*SC)
        xt = np.ascontiguousarray(
            hidden[b][r * SC : (r + 1) * SC]
            .reshape(SC, 4, DKT // 4, 128)
            .transpose(1, 3, 2, 0)
        ).astype(bfloat16)  # [ch, p, kt, s]
        cosq = np.ascontiguousarray(cosk[:, qpos])
        sinq = np.ascontiguousarray(sink[:, qpos])
        cosk_own = np.ascontiguousarray(cosk[:, r * SC : (r + 1) * SC])
        sink_own = np.ascontiguousarray(sink[:, r * SC : (r + 1) * SC])
        # mask[kt]: [128, 128] TRANSPOSED additive mask ([q, k] layout — it is
        # the stationary operand of a += maskT^T @ I accumulate on the PE) for
        # the FIRST live block of the suffix (columns QW-w .. QW-w+128).
        # Triangular when that block's q-tile equals kt (the diagonal),
        # all -inf when the block is non-causal (dead), all-zero otherwise.
        mask = np.zeros((NKT, 128, 128), np.float32)
        for kti in range(NKT):
            lo = QW - _wof(kti)
            kk = kti * 128 + np.arange(128)[None, :]
            qq = qpos[lo : lo + 128, None]
            mask[kti] = np.where(kk <= qq, 0.0, -30000.0)
        mask = mask.astype(bfloat16)
        in_maps.append(
            {
                "xq": xq,
                "xt": xt,
                "wq": wq,
                "wkv": wkv,
                "wo": wo,
                "cosq": cosq,
                "sinq": sinq,
                "cosk": cosk_own,
                "sink": sink_own,
                "mask": mask,
            }
        )
    return in_maps


def kernel(hidden_states, Wq, Wk, Wv, Wo, trace=False):
    nc = _build_nc()
    in_maps = _host_inputs(hidden_states, Wq, Wk, Wv, Wo)
    res = run_bass_kernel_spmd(nc, in_maps, list(range(8)), trace=trace)
    out = np.empty((B, S, D), dtype=np.float32)
    for core in range(8):
        b, r = core // 4, core % 4
        o = np.asarray(res.results[core]["out"], dtype=np.float32)
        for j, t0 in enumerate(_qtiles(r)):
            out[b, t0 * 128 : (t0 + 1) * 128, :] = o[j]
    if trace:
        kernel.last_exec_time_ns = res.exec_time_ns
    return out


# revision 33
# speedup vs baseline: 1.0260x; 1.0108x over previous
"""DreamAttention (GQA + RoPE + causal) on 8 trn2 NeuronCores.

Sharding: DP=2 over batch x sequence-parallel over q-tiles. Core c ->
(batch b = c // 4, seq rank r = c % 4). Core r owns q-tiles
[r, 7-r, 8+r, 15-r] (128 rows each, ascending) — every core gets exactly 34
k-tile-blocks of causal attention work, so the load is perfectly balanced.

K/V projection is seq-sharded: each core computes K^T/V for ONLY its own
512 contiguous positions (1/4 of S), then an AllGather over the 4-core
batch group assembles the full K/V. The collective runs on TOPSP+SDMA
silicon; it is sandwiched between the two halves of the Q projection so
its SDMA traffic overlaps PE work whose weights are already buffered
(wqp ring depth 6). This removes the 4x redundant K/V compute (~150us of
PE time) a collective-free version would pay.

All matmul operands are bf16 (fp32 PSUM accumulation). The first ~230us
is simultaneously PE- and DMA-bound (~51MB of weights/activations at
~240GB/s effective), so DMA queue ORDER is tuned: sync carries A0's
weights then Wo (ring-throttled into phase C), scalar carries xt-half +
xq + the wq ring, gpsimd carries the collective + gather-back. Host-side
layouts give every big stream >=7KB contiguous per-partition lines.

Per-core dataflow:
  - A1 heads 0-13: Q projection + fused Q-RoPE -> qt[:, h, :].
  - A0: K/V projection for own 512 positions (K-RoPE fused), V
    PE-transposed to natural layout; chunks to internal DRAM; AllGather
    [[0-3],[4-7]]; gather back into kt_t [128, 4kv, S] / vn.
  - A1 heads 14-27 (collective in flight underneath).
  - B: attention per (head, seg): k-tiles are packed into 6 two-bank
    [128,1024] PSUM segments so exp runs as 6 wide ACTIVATEs instead of
    16 narrow ones (the ACT engine costs ~200ns/op + w/1.2GHz; this cut
    exp from 7.5 to ~5.5us/head). Matmul outputs never cross a 2KB PSUM
    bank boundary (the 384-wide pairs sit at offsets 0/512). The additive
    causal mask for the first live 128-block of each k-tile rides the PE
    as a += maskT^T @ I accumulate — keeping it on-PE keeps the PE dense
    enough that HAM stays at K=8/8 (off-PE masking measurably dropped the
    PE clock). P^T -> exp -> PV with a one-segment software-pipeline lag;
    the softmax denominator (ones-matmul partition reduce + PE
    outer-product broadcast + normalize) for head h is emitted INSIDE
    head h+1's segment loop so the PE FIFO never stalls on the DVE chain.
  - C: o_proj (full Wo); attnT stationary, Wo moving, accumulate over 28
    head-chunks; output rows are core-owned -> DMA straight out as bf16.
Host reassembles the 8 cores' row-slices into the full [2, 2048, 3584] output.
"""

import math

import numpy as np
from ml_dtypes import bfloat16

import concourse.bass as bass
import concourse.mybir as mybir
import concourse.tile as tile
from concourse import bacc
from concourse.bass_utils import run_bass_kernel_spmd
from concourse.masks import make_identity

F32 = mybir.dt.float32
BF16 = mybir.dt.bfloat16

B, S, D = 2, 2048, 3584
H, KVH, HD = 28, 4, 128
ROPE_THETA = 1000000.0
GQ = H // KVH   # 7 q heads per kv head
DKT = D // 128  # 28 k-tiles over D
SC = 512        # per-core owned K/V chunk (S / 4)
NKT = S // 128  # 16 k tiles over sequence
NST = SC // 128  # 4 seq tiles per owned chunk
NDC = 7         # output D chunks of 512
NQT = 4         # q-tiles owned per core
QW = NQT * 128  # 512 q columns per core
SCALE = 1.0 / math.sqrt(HD)
PVDEPTH = 3     # attention software-pipeline depth (S runs ahead of PV)
RG = [[0, 1, 2, 3], [4, 5, 6, 7]]  # batch groups for the K/V AllGather


def _qtiles(r):
    """Ascending q-tile ids owned by seq-rank r; sum of (t+1) == 34 for all r."""
    return [r, 7 - r, 8 + r, 15 - r]


def _wof(kti):
    # Live-suffix width for k-tile kti. Rank-independent: every rank's
    # ascending tile list [t0<t1<t2<t3] satisfies t0<=3, 4<=t1<=7, 8<=t2<=11,
    # 12<=t3<=15, so #(tiles >= kti) == 4 - kti//4 for all ranks.
    return 128 * (4 - kti // 4)


_NC_CACHE = {}


def _build_nc():
    key = "nc"
    if key in _NC_CACHE:
        return _NC_CACHE[key]

    nc = bacc.Bacc("TRN2", target_bir_lowering=False, debug=False, num_devices=8)

    xq_d = nc.dram_tensor("xq", [4, 128, DKT // 4, QW], BF16, kind="ExternalInput").ap()
    xt_d = nc.dram_tensor("xt", [4, 128, DKT // 4, SC], BF16, kind="ExternalInput").ap()
    wq_d = nc.dram_tensor("wq", [H, 128, DKT, 128], BF16, kind="ExternalInput").ap()
    wkv_d = nc.dram_tensor(
        "wkv", [2 * KVH, 2, 128, DKT // 2, 128], BF16, kind="ExternalInput"
    ).ap()
    wo_d = nc.dram_tensor(
        "wo", [NDC, 4, 128, DKT // 4, 512], BF16, kind="ExternalInput"
    ).ap()
    cosq_d = nc.dram_tensor("cosq", [128, QW], BF16, kind="ExternalInput").ap()
    sinq_d = nc.dram_tensor("sinq", [128, QW], BF16, kind="ExternalInput").ap()
    cosk_d = nc.dram_tensor("cosk", [128, SC], BF16, kind="ExternalInput").ap()
    sink_d = nc.dram_tensor("sink", [128, SC], BF16, kind="ExternalInput").ap()
    mask_d = nc.dram_tensor("mask", [128, NKT, 128], BF16, kind="ExternalInput").ap()
    out_d = nc.dram_tensor("out", [NQT, 128, D], BF16, kind="ExternalOutput").ap()

    # K/V AllGather bounce buffers (collectives can't touch kernel I/O).
    # cc_in[0] = own K^T chunk [128, 4kv, 512pos]; cc_in[1] = own V natural
    # chunk [128pos-in-tile, 4kv, 4tile, 128d]. AllGather concatenates rank
    # shards along the leading axis of cc_out.
    cc_in = nc.dram_tensor("cc_in", [2, 128, 2048], BF16)
    # Shared addr_space needs >4-core groups; Local works for 4-core AG.
    cc_out = nc.dram_tensor("cc_out", [4, 2, 128, 2048], BF16)

    with tile.TileContext(nc) as tc:
        with tc.tile_pool(name="persist", bufs=1) as persist:
            # qt doubles as the attention-output buffer: att(h) overwrites
            # qt[:, h, :] once head h's scores are done.
            qt = persist.tile([128, H, QW], BF16, name="qt")
            kt_t = persist.tile([128, KVH, S], BF16, name="kt")
            vn = persist.tile([128, KVH, NKT, 128], BF16, name="vn")
            ident = persist.tile([128, 128], BF16, name="ident")
            mask_t = persist.tile([128, NKT, 128], BF16, name="mask_t")
            ones = persist.tile([128, 1], BF16, name="ones")
            ones_row = persist.tile([1, 128], BF16, name="ones_row")

            make_identity(nc, ident)
            nc.vector.memset(ones, 1.0)
            nc.vector.memset(ones_row, 1.0)

            # dependency-free warmup matmuls: bridge the ~10us of input-DMA
            # wait at kernel start AND hold the PE-HAM activity window so
            # the first real matmuls run at K=8/8 instead of half clock.
            with tc.tile_pool(name="pwarm", bufs=1, space="PSUM") as pwarm:
                wps = pwarm.tile([128, 128], F32, name="wps")
                for i in range(100):
                    nc.tensor.matmul(
                        wps, ident, ident, start=(i == 0), stop=(i == 99)
                    )

            def rope(dst, cos_ap, sin_ap, width, tmp, eng):
                # eng: DMA queue for the rotate-half copies. Phase A1 must
                # NOT use gpsimd — its engine stream blocks in the
                # collective's wait_ge and would stall A1's DVE chain.
                t = tmp[:, :width]
                eng.dma_start(out=t[0:64, :], in_=dst[64:128, :])
                eng.dma_start(out=t[64:128, :], in_=dst[0:64, :])
                nc.vector.tensor_mul(t, t, sin_ap)
                nc.vector.tensor_mul(dst, dst, cos_ap)
                nc.vector.tensor_add(dst, dst, t)

            # ---- Phase A0: K/V projection for OWN 512 positions, K-RoPE
            # fused; then AllGather across the 4-core batch group.
            # ---- Phase A1: Q projection + fused Q-RoPE (AllGather hides
            # under this).
            # One pool scope for both phases: separate scopes would make
            # A1's xq/wq prefetch wait for A0's SBUF to release (measured
            # ~38us of PE stall at the seam).
            with (
                tc.tile_pool(name="ropetab", bufs=1) as ropetab,
                tc.tile_pool(name="ropep", bufs=2) as ropep,
                tc.tile_pool(name="xtp", bufs=4) as xtp,
                tc.tile_pool(name="wkvp", bufs=6) as wkvp,
                tc.tile_pool(name="kvchunk", bufs=1) as kvchunk,
                tc.tile_pool(name="xqp", bufs=1) as xqp,
                tc.tile_pool(name="wqp", bufs=6) as wqp,
                tc.tile_pool(name="qtab", bufs=1) as qtab,
                tc.tile_pool(name="qrtmp", bufs=2) as qrtmp,
                tc.tile_pool(name="ps_kv", bufs=2, space="PSUM") as ps_kv,
                tc.tile_pool(name="ps_tr", bufs=2, space="PSUM") as ps_tr,
                tc.tile_pool(name="ps_a", bufs=3, space="PSUM") as ps_a,
            ):
                # DMA issue order tuned so the first matmul starts ~3us in:
                # wkv[0,0] leads the sync queue, xt[0] right behind it.
                wkv_tiles = {}

                def load_wkv(ct):
                    # all K/V weights on sync: the scalar queue belongs to
                    # the xq/wq stream, whose pool ring-waits must never
                    # sit AHEAD of A0's weight refills.
                    for hf in range(2):
                        wblk = wkvp.tile([128, DKT // 2, 128], BF16, name="wkv")
                        nc.sync.dma_start(out=wblk, in_=wkv_d[ct, hf])
                        wkv_tiles[(ct, hf)] = wblk

                cosk = ropetab.tile([128, SC], BF16, name="cosk")
                sink = ropetab.tile([128, SC], BF16, name="sink")
                nc.scalar.dma_start(out=cosk, in_=cosk_d)
                nc.scalar.dma_start(out=sink, in_=sink_d)
                load_wkv(0)
                xts = []
                for ch in range(4):
                    xtile = xtp.tile([128, DKT // 4, SC], BF16, name="xt")
                    eng = nc.sync if ch < 2 else nc.scalar
                    eng.dma_start(out=xtile, in_=xt_d[ch])
                    for j in range(DKT // 4):
                        xts.append(xtile[:, j, :])
                for ct in range(1, 3):
                    load_wkv(ct)
                # A1 prefetch: xq + first wq blocks stream on the SCALAR
                # queue behind A0's xt half, so they fill during A0's
                # compute without ever delaying A0's sync-side weights.
                cosq = qtab.tile([128, QW], BF16, name="cosq")
                sinq = qtab.tile([128, QW], BF16, name="sinq")
                nc.scalar.dma_start(out=cosq, in_=cosq_d)
                nc.scalar.dma_start(out=sinq, in_=sinq_d)
                xq = xqp.tile([128, DKT, QW], BF16, name="xq")
                for ch in range(4):
                    nc.scalar.dma_start(
                        out=xq[:, ch * (DKT // 4) : (ch + 1) * (DKT // 4), :],
                        in_=xq_d[ch],
                    )
                # mask table early (contiguous, host pre-transposed): B's
                # first mask-matmul must not wait on the blocked gpsimd queue
                nc.scalar.dma_start(out=mask_t, in_=mask_d)
                wq_tiles = {}

                def load_wq(ct):
                    wblk = wqp.tile([128, DKT, 128], BF16, name="wq")
                    nc.scalar.dma_start(out=wblk, in_=wq_d[ct])
                    wq_tiles[ct] = wblk

                for ct in range(6):
                    load_wq(ct)

                def a1_heads(h_lo, h_hi):
                    # Q projection + fused Q-RoPE for heads [h_lo, h_hi)
                    for ct in range(h_lo, h_hi):
                        if ct + 6 < H:
                            load_wq(ct + 6)
                        wblk = wq_tiles.pop(ct)
                        psum = ps_a.tile([128, QW], F32, name="pp")
                        for kti in range(DKT):
                            nc.tensor.matmul(
                                psum,
                                wblk[:, kti, :],
                                xq[:, kti, :],
                                start=(kti == 0),
                                stop=(kti == DKT - 1),
                            )
                        nc.vector.tensor_copy(qt[:, ct, :], psum)
                        tmp = qrtmp.tile([128, QW], BF16, name="qrtmp")
                        rope(qt[:, ct, :], cosq, sinq, QW, tmp, nc.scalar)

                # A1 front half FIRST: the uncontended early DMA window
                # feeds xq + the wq ring; A0 + the AllGather run in the
                # middle so the collective's SDMA traffic overlaps A1's
                # back half, whose weights are already buffered (wqp=6).
                a1_heads(0, 14)

                kt_own = kvchunk.tile([128, KVH, SC], BF16, name="kt_own")
                vt_own = kvchunk.tile([128, KVH, SC], BF16, name="vt_own")
                vn_own = kvchunk.tile([128, KVH, NST, 128], BF16, name="vn_own")
                for ct in range(2 * KVH):  # 0-3: K heads, 4-7: V
                    if ct + 3 < 2 * KVH:
                        load_wkv(ct + 3)
                    psum = ps_kv.tile([128, SC], F32, name="pp")
                    for hf in range(2):
                        wblk = wkv_tiles.pop((ct, hf))
                        for kti in range(DKT // 2):
                            gkt = hf * (DKT // 2) + kti
                            nc.tensor.matmul(
                                psum,
                                wblk[:, kti, :],
                                xts[gkt],
                                start=(gkt == 0),
                                stop=(gkt == DKT - 1),
                            )
                    if ct < KVH:
                        nc.vector.tensor_copy(kt_own[:, ct, :], psum)
                        tmp = ropep.tile([128, SC], BF16, name="ropetmp")
                        rope(kt_own[:, ct, :], cosk, sink, SC, tmp, nc.gpsimd)
                    else:
                        nc.vector.tensor_copy(vt_own[:, ct - KVH, :], psum)
                # V^T -> V natural (4 s-tiles x 4 heads)
                for kv in range(KVH):
                    for sti in range(NST):
                        ptr = ps_tr.tile([128, 128], BF16, name="ptr")
                        nc.tensor.transpose(
                            ptr,
                            vt_own[:, kv, sti * 128 : (sti + 1) * 128],
                            ident,
                        )
                        nc.vector.tensor_copy(vn_own[:, kv, sti, :], ptr)
                # own chunks -> DRAM bounce, AllGather, gather back
                nc.gpsimd.dma_start(
                    out=cc_in[0].rearrange("p (kv s) -> p kv s", kv=KVH),
                    in_=kt_own,
                )
                nc.gpsimd.dma_start(
                    out=cc_in[1].rearrange(
                        "p (kv st d) -> p kv st d", kv=KVH, st=NST
                    ),
                    in_=vn_own,
                )
                nc.gpsimd.collective_compute(
                    "AllGather",
                    mybir.AluOpType.bypass,
                    replica_groups=RG,
                    ins=[cc_in.ap()],
                    outs=[cc_out.ap()],
                )
                # gather-back on gpsimd: it already blocks in the
                # collective's wait_ge, and using sync here would
                # head-of-line-block phase A1's weight loads behind the
                # collective (measured: ~50us PE stall).
                for rr in range(4):
                    nc.gpsimd.dma_start(
                        out=kt_t[:, :, rr * SC : (rr + 1) * SC],
                        in_=cc_out[rr, 0].rearrange("p (kv s) -> p kv s", kv=KVH),
                    )
                    nc.gpsimd.dma_start(
                        out=vn[:, :, rr * NST : (rr + 1) * NST, :],
                        in_=cc_out[rr, 1].rearrange(
                            "p (kv st d) -> p kv st d", kv=KVH, st=NST
                        ),
                    )

                # ---- A1 back half (collective in flight underneath) ----
                a1_heads(14, H)

            # ---- Phase B+C share the wop pool so Wo prefetches during B ----
            with (
                tc.tile_pool(name="wop", bufs=8) as wop,
                tc.tile_pool(name="outp", bufs=3) as outp,
            ):
                wo_tiles = {}

                def load_wo(dc):
                    # 4 ring-throttled chunk DMAs per dc: the ring keeps the
                    # bulk of Wo streaming during phase C (not during the
                    # collective window, which a single early DMA would hit).
                    tl = []
                    for ch in range(4):
                        wt = wop.tile([128, DKT // 4, 512], BF16, name="wo")
                        nc.sync.dma_start(out=wt, in_=wo_d[dc, ch])
                        tl.append(wt)
                    wo_tiles[dc] = tl

                load_wo(0)
                load_wo(1)

                # ---- Phase B: attention, 28 heads on the core's 512 q ----
                # k-tiles are packed into 6 PSUM "segments" per head (each a
                # 2-bank [128, 1024] f32 tile) so exp runs as 6 wide
                # ACTIVATEs instead of 16 narrow ones — the scalar engine's
                # ~200ns/op overhead made exp the phase bottleneck (7.5us
                # -> ~5.5us per head). The 0/1 causal-mask multiplies run on
                # GpSimd (otherwise idle); denominator accumulation stays on
                # DVE.  SEGS: list of ([(kti, column offset)...], exp width).
                # A matmul output must NOT cross a 2KB PSUM bank boundary,
                # so the 384-wide pairs sit at offsets 0/512 and exp spans
                # the (unread) garbage hole in between.
                SEGS = [
                    ([(0, 0), (1, 512)], 1024),
                    ([(2, 0), (3, 512)], 1024),
                    ([(4, 0), (5, 512)], 896),
                    ([(6, 0), (7, 512)], 896),
                    ([(8, 0), (9, 256), (10, 512), (11, 768)], 1024),
                    ([(12, 0), (13, 128), (14, 256), (15, 384)], 512),
                ]
                with (
                    tc.tile_pool(name="ptp", bufs=3) as ptp,
                    tc.tile_pool(name="accp", bufs=2) as accp,
                    tc.tile_pool(name="smallp", bufs=4) as smallp,
                    tc.tile_pool(name="ps_s", bufs=2, space="PSUM") as ps_s,
                    tc.tile_pool(name="ps_o", bufs=2, space="PSUM") as ps_o,
                    tc.tile_pool(name="ps_r", bufs=1, space="PSUM") as ps_r,
                ):
                    # deferred softmax-denominator pipeline: emit head h-1's
                    # partition-reduce / reciprocal / broadcast / normalize
                    # interleaved into head h's segment loop so the PE FIFO
                    # never stalls on the DVE accumulation chain.
                    pend_norm = []

                    def norm_step(step):
                        if not pend_norm:
                            return
                        if step == 0:
                            ph, pacc, ppo = pend_norm[0]
                            # denominator: single partition-reduce matmul
                            psum_r = ps_r.tile([1, QW], F32, name="psr")
                            nc.tensor.matmul(
                                psum_r, ones, pacc, start=True, stop=True
                            )
                            rec = smallp.tile([1, QW], F32, name="rec")
                            nc.vector.reciprocal_approx_fast(rec, psum_r)
                            rec_bf = smallp.tile([1, QW], BF16, name="rec_bf")
                            nc.vector.tensor_copy(rec_bf, rec)
                            pend_norm[0] = (ph, pacc, ppo, rec_bf)
                        elif step == 1:
                            ph, pacc, ppo, rec_bf = pend_norm[0]
                            # broadcast along partitions via PE outer product
                            # (DMA cannot do stride-0 partition reads)
                            bcast = ps_r.tile([128, QW], F32, name="bcast")
                            nc.tensor.matmul(
                                bcast, ones_row, rec_bf, start=True, stop=True
                            )
                            bcast_sb = smallp.tile([128, QW], F32, name="bcast_sb")
                            nc.vector.tensor_copy(bcast_sb, bcast)
                            pend_norm[0] = (ph, pacc, ppo, bcast_sb)
                        else:
                            ph, pacc, ppo, bcast_sb = pend_norm.pop(0)
                            # fused normalize + PSUM->SBUF, overwriting head
                            # ph's spent q columns
                            nc.vector.tensor_mul(qt[:, ph, :], ppo, bcast_sb)

                    for h in range(H):
                        kv = h // GQ
                        psum_o = ps_o.tile([128, QW], F32, name="po")
                        acc = accp.tile([128, QW], BF16, name="acc")
                        pend = []

                        def emit_pv(ent, psum_o=psum_o, kv=kv):
                            kti, lo, w, pt, off = ent
                            nc.tensor.matmul(
                                psum_o[:, lo : lo + w],
                                vn[:, kv, kti, :],
                                pt[:, off : off + w],
                                start=(kti == 0),
                                stop=(kti == NKT - 1),
                            )

                        for si, (ktis, segw) in enumerate(SEGS):
                            w = _wof(ktis[0][0])
                            lo = QW - w
                            pseg = ps_s.tile([128, 1024], F32, name="pss")
                            for kti, off in ktis:
                                nc.tensor.matmul(
                                    pseg[:, off : off + w],
                                    kt_t[:, kv, kti * 128 : (kti + 1) * 128],
                                    qt[:, h, lo:],
                                    start=True,
                                    stop=False,
                                )
                                # additive causal mask folded into the score
                                # accumulation group on the PE itself: the
                                # first live 128-block (the only one that can
                                # be diagonal or dead) gets += maskT^T @ I.
                                # Cheapest engine per op (~107ns, LDW-bound),
                                # and the extra PE density keeps HAM at
                                # K=8/8 — off-PE masking measurably dropped
                                # the PE clock to ~1.4GHz in phase B.
                                nc.tensor.matmul(
                                    pseg[:, off : off + 128],
                                    mask_t[:, kti, :],
                                    ident,
                                    start=False,
                                    stop=True,
                                )
                            pt = ptp.tile([128, 1024], BF16, name="pt")
                            nc.scalar.activation(
                                pt[:, :segw],
                                pseg[:, :segw],
                                mybir.ActivationFunctionType.Exp,
                            )
                            for kti, off in ktis:
                                # running softmax-denominator partial on DVE
                                if kti == 0:
                                    nc.vector.tensor_copy(
                                        acc, pt[:, off : off + w]
                                    )
                                else:
                                    nc.vector.tensor_add(
                                        acc[:, lo:],
                                        acc[:, lo:],
                                        pt[:, off : off + w],
                                    )
                                pend.append((kti, lo, w, pt, off))
                            # one-segment lag: PVs of segment si-1 emit here
                            while len(pend) > len(ktis):
                                emit_pv(pend.pop(0))
                            if si == 1:
                                norm_step(0)
                            elif si == 2:
                                norm_step(1)
                            elif si == 3:
                                norm_step(2)
                        while pend:
                            emit_pv(pend.pop(0))
                        pend_norm.append((h, acc, psum_o))
                    # drain the last head's normalization
                    norm_step(0)
                    norm_step(1)
                    norm_step(2)

                # ---- Phase C: o_proj (full Wo); rows are core-owned ----
                with tc.tile_pool(name="ps_c", bufs=2, space="PSUM") as ps_c:
                    for dc in range(NDC):
                        if dc + 2 < NDC:
                            load_wo(dc + 2)
                        tl = wo_tiles.pop(dc)
                        for q in range(NQT):
                            psum = ps_c.tile([128, 512], F32, name="pp")
                            for ct in range(DKT):
                                nc.tensor.matmul(
                                    psum,
                                    qt[:, ct, q * 128 : (q + 1) * 128],
                                    tl[ct // 7][:, ct % 7, :],
                                    start=(ct == 0),
                                    stop=(ct == DKT - 1),
                                )
                            ob = outp.tile([128, 512], BF16, name="ob")
                            nc.vector.tensor_copy(ob, psum)
                            nc.scalar.dma_start(
                                out=out_d[q, :, dc * 512 : (dc + 1) * 512],
                                in_=ob,
                            )

    nc.finalize()
    _NC_CACHE[key] = nc
    return nc


def _host_inputs(hidden_states, Wq, Wk, Wv, Wo):
    hidden = np.asarray(hidden_states, dtype=np.float32)
    Wq = np.asarray(Wq, dtype=np.float32) * np.float32(SCALE)
    Wk = np.asarray(Wk, dtype=np.float32)
    Wv = np.asarray(Wv, dtype=np.float32)
    Wo = np.asarray(Wo, dtype=np.float32)

    inv_freq = 1.0 / ROPE_THETA ** (np.arange(0, HD, 2, dtype=np.float32) / HD)
    t = np.arange(S, dtype=np.float32)
    freqs = np.outer(t, inv_freq)  # [S, 64]
    cos_t = np.cos(freqs.T)  # [64, S]
    sin_t = np.sin(freqs.T)
    cosk = np.concatenate([cos_t, cos_t], axis=0).astype(bfloat16)  # [128, S]
    sink = np.concatenate([-sin_t, sin_t], axis=0).astype(bfloat16)

    # shared weight layouts (identical for every core)
    wq = np.ascontiguousarray(
        Wq.reshape(DKT, 128, H, 128).transpose(2, 1, 0, 3)
    ).astype(bfloat16)  # [h, p, kt, c]
    wk4 = Wk.reshape(DKT, 128, KVH, 128)
    wv4 = Wv.reshape(DKT, 128, KVH, 128)
    wkv = np.empty((2 * KVH, 2, 128, DKT // 2, 128), np.float32)
    for ct in range(KVH):
        for hf in range(2):
            ktsl = slice(hf * (DKT // 2), (hf + 1) * (DKT // 2))
            wkv[ct, hf] = wk4[ktsl, :, ct, :].transpose(1, 0, 2)
            wkv[KVH + ct, hf] = wv4[ktsl, :, ct, :].transpose(1, 0, 2)
    wkv = wkv.astype(bfloat16)
    wo = np.ascontiguousarray(
        Wo.reshape(4, DKT // 4, 128, NDC, 512).transpose(3, 0, 2, 1, 4)
    ).astype(bfloat16)  # [dc, ch, p, kt, d]

    in_maps = []
    for core in range(8):
        b, r = core // 4, core % 4
        tiles = _qtiles(r)
        qpos = np.concatenate(
            [np.arange(t0 * 128, (t0 + 1) * 128) for t0 in tiles]
        )  # [512] ascending global q positions
        xq = np.ascontiguousarray(
            hidden[b][qpos].reshape(QW, 4, DKT // 4, 128).transpose(1, 3, 2, 0)
        ).astype(bfloat16)  # [ch, p, kt, q]
        # own contiguous K/V chunk: positions [r*SC, (r+1)*SC)
        xt = np.ascontiguousarray(
            hidden[b][r * SC : (r + 1) * SC]
            .reshape(SC, 4, DKT // 4, 128)
            .transpose(1, 3, 2, 0)
        ).astype(bfloat16)  # [ch, p, kt, s]
        cosq = np.ascontiguousarray(cosk[:, qpos])
        sinq = np.ascontiguousarray(sink[:, qpos])
        cosk_own = np.ascontiguousarray(cosk[:, r * SC : (r + 1) * SC])
        sink_own = np.ascontiguousarray(sink[:, r * SC : (r + 1) * SC])
        # mask[kt]: [128, 128] TRANSPOSED additive mask ([q, k] layout — it is
        # the stationary operand of a += maskT^T @ I accumulate on the PE) for
        # the FIRST live block of the suffix (columns QW-w .. QW-w+128).
        # Triangular when that block's q-tile equals kt (the diagonal),
        # all -inf when the block is non-causal (dead), all-zero otherwise.
        mask = np.zeros((NKT, 128, 128), np.float32)
        for kti in range(NKT):
            lo = QW - _wof(kti)
            kk = kti * 128 + np.arange(128)[None, :]
            qq = qpos[lo : lo + 128, None]
            mask[kti] = np.where(kk <= qq, 0.0, -30000.0)
        # pre-transposed to the on-chip [q-part, kt, k] layout so the DMA is
        # one contiguous 4KB-line stream instead of a 256B-line gather
        mask = np.ascontiguousarray(mask.transpose(1, 0, 2)).astype(bfloat16)
        in_maps.append(
            {
                "xq": xq,
                "xt": xt,
                "wq": wq,
                "wkv": wkv,
                "wo": wo,
                "cosq": cosq,
                "sinq": sinq,
                "cosk": cosk_own,
                "sink": sink_own,
                "mask": mask,
            }
        )
    return in_maps


def kernel(hidden_states, Wq, Wk, Wv, Wo, trace=False):
    nc = _build_nc()
    in_maps = _host_inputs(hidden_states, Wq, Wk, Wv, Wo)
    res = run_bass_kernel_spmd(nc, in_maps, list(range(8)), trace=trace)
    out = np.empty((B, S, D), dtype=np.float32)
    for core in range(8):
        b, r = core // 4, core % 4
        o = np.asarray(res.results[core]["out"], dtype=np.float32)
        for j, t0 in enumerate(_qtiles(r)):
            out[b, t0 * 128 : (t0 + 1) * 128, :] = o[j]
    if trace:
        kernel.last_exec_time_ns = res.exec_time_ns
    return out


# revision 34
# speedup vs baseline: 1.0503x; 1.0236x over previous
"""DreamAttention (GQA + RoPE + causal) on 8 trn2 NeuronCores.

Sharding: DP=2 over batch x sequence-parallel over q-tiles. Core c ->
(batch b = c // 4, seq rank r = c % 4). Core r owns q-tiles
[r, 7-r, 8+r, 15-r] (128 rows each, ascending) — every core gets exactly 34
k-tile-blocks of causal attention work, so the load is perfectly balanced.

K/V projection is seq-sharded: each core computes K^T/V for ONLY its own
512 contiguous positions (1/4 of S), then an AllGather over the 4-core
batch group assembles the full K/V. The collective runs on TOPSP+SDMA
silicon; it is sandwiched between the two halves of the Q projection so
its SDMA traffic overlaps PE work whose weights are already buffered
(wqp ring depth 6). This removes the 4x redundant K/V compute (~150us of
PE time) a collective-free version would pay.

All matmul operands are bf16 (fp32 PSUM accumulation). The first ~230us
is simultaneously PE- and DMA-bound (~51MB of weights/activations at
~240GB/s effective), so DMA queue ORDER is tuned: sync carries A0's
weights then Wo (ring-throttled into phase C), scalar carries xt-half +
xq + the wq ring, gpsimd carries the collective + gather-back. Host-side
layouts give every big stream >=7KB contiguous per-partition lines.

Per-core dataflow:
  - A1 heads 0-13: Q projection + fused Q-RoPE -> qt[:, h, :].
  - A0: K/V projection for own 512 positions (K-RoPE fused), V
    PE-transposed to natural layout; chunks to internal DRAM; AllGather
    [[0-3],[4-7]]; gather back into kt_t [128, 4kv, S] / vn.
  - A1 heads 14-27 (collective in flight underneath).
  - B: attention per (head, seg): k-tiles are packed into 6 two-bank
    [128,1024] PSUM segments so exp runs as 6 wide ACTIVATEs instead of
    16 narrow ones (the ACT engine costs ~200ns/op + w/1.2GHz; this cut
    exp from 7.5 to ~5.5us/head). Matmul outputs never cross a 2KB PSUM
    bank boundary (the 384-wide pairs sit at offsets 0/512). The additive
    causal mask for the first live 128-block of each k-tile rides the PE
    as a += maskT^T @ I accumulate — keeping it on-PE keeps the PE dense
    enough that HAM stays at K=8/8 (off-PE masking measurably dropped the
    PE clock). P^T -> exp -> PV with a one-segment software-pipeline lag;
    the softmax denominator (ones-matmul partition reduce + PE
    outer-product broadcast + normalize) for head h is emitted INSIDE
    head h+1's segment loop so the PE FIFO never stalls on the DVE chain.
  - C: o_proj (full Wo); attnT stationary, Wo moving, accumulate over 28
    head-chunks; output rows are core-owned -> DMA straight out as bf16.
Host reassembles the 8 cores' row-slices into the full [2, 2048, 3584] output.
"""

import math

import numpy as np
from ml_dtypes import bfloat16

import concourse.bass as bass
import concourse.mybir as mybir
import concourse.tile as tile
from concourse import bacc
from concourse.bass_utils import run_bass_kernel_spmd
from concourse.masks import make_identity

F32 = mybir.dt.float32
BF16 = mybir.dt.bfloat16

B, S, D = 2, 2048, 3584
H, KVH, HD = 28, 4, 128
ROPE_THETA = 1000000.0
GQ = H // KVH   # 7 q heads per kv head
DKT = D // 128  # 28 k-tiles over D
SC = 512        # per-core owned K/V chunk (S / 4)
NKT = S // 128  # 16 k tiles over sequence
NST = SC // 128  # 4 seq tiles per owned chunk
NDC = 7         # output D chunks of 512
NQT = 4         # q-tiles owned per core
QW = NQT * 128  # 512 q columns per core
SCALE = 1.0 / math.sqrt(HD)
PVDEPTH = 3     # attention software-pipeline depth (S runs ahead of PV)
RG = [[0, 1, 2, 3], [4, 5, 6, 7]]  # batch groups for the K/V AllGather


def _qtiles(r):
    """Ascending q-tile ids owned by seq-rank r; sum of (t+1) == 34 for all r."""
    return [r, 7 - r, 8 + r, 15 - r]


def _wof(kti):
    # Live-suffix width for k-tile kti. Rank-independent: every rank's
    # ascending tile list [t0<t1<t2<t3] satisfies t0<=3, 4<=t1<=7, 8<=t2<=11,
    # 12<=t3<=15, so #(tiles >= kti) == 4 - kti//4 for all ranks.
    return 128 * (4 - kti // 4)


_NC_CACHE = {}


def _build_nc():
    key = "nc"
    if key in _NC_CACHE:
        return _NC_CACHE[key]

    nc = bacc.Bacc("TRN2", target_bir_lowering=False, debug=False, num_devices=8)

    xq_d = nc.dram_tensor("xq", [4, 128, DKT // 4, QW], BF16, kind="ExternalInput").ap()
    xt_d = nc.dram_tensor("xt", [4, 128, DKT // 4, SC], BF16, kind="ExternalInput").ap()
    wq_d = nc.dram_tensor("wq", [H, 128, DKT, 128], BF16, kind="ExternalInput").ap()
    wkv_d = nc.dram_tensor(
        "wkv", [2 * KVH, 2, 128, DKT // 2, 128], BF16, kind="ExternalInput"
    ).ap()
    wo_d = nc.dram_tensor(
        "wo", [NDC, 4, 128, DKT // 4, 512], BF16, kind="ExternalInput"
    ).ap()
    cosq_d = nc.dram_tensor("cosq", [128, QW], BF16, kind="ExternalInput").ap()
    sinq_d = nc.dram_tensor("sinq", [128, QW], BF16, kind="ExternalInput").ap()
    cosk_d = nc.dram_tensor("cosk", [128, SC], BF16, kind="ExternalInput").ap()
    sink_d = nc.dram_tensor("sink", [128, SC], BF16, kind="ExternalInput").ap()
    mask_d = nc.dram_tensor("mask", [128, NKT, 128], BF16, kind="ExternalInput").ap()
    out_d = nc.dram_tensor("out", [NQT, 128, D], BF16, kind="ExternalOutput").ap()

    # K/V AllGather bounce buffers (collectives can't touch kernel I/O).
    # cc_in[0] = own K^T chunk [128, 4kv, 512pos]; cc_in[1] = own V natural
    # chunk [128pos-in-tile, 4kv, 4tile, 128d]. AllGather concatenates rank
    # shards along the leading axis of cc_out.
    cc_in = nc.dram_tensor("cc_in", [2, 128, 2048], BF16)
    # Shared addr_space needs >4-core groups; Local works for 4-core AG.
    cc_out = nc.dram_tensor("cc_out", [4, 2, 128, 2048], BF16)

    with tile.TileContext(nc) as tc:
        with tc.tile_pool(name="persist", bufs=1) as persist:
            # qt doubles as the attention-output buffer: att(h) overwrites
            # qt[:, h, :] once head h's scores are done.
            qt = persist.tile([128, H, QW], BF16, name="qt")
            kt_t = persist.tile([128, KVH, S], BF16, name="kt")
            vn = persist.tile([128, KVH, NKT, 128], BF16, name="vn")
            ident = persist.tile([128, 128], BF16, name="ident")
            mask_t = persist.tile([128, NKT, 128], BF16, name="mask_t")
            ones = persist.tile([128, 1], BF16, name="ones")
            ones_row = persist.tile([1, 128], BF16, name="ones_row")

            make_identity(nc, ident)
            nc.vector.memset(ones, 1.0)
            nc.vector.memset(ones_row, 1.0)

            # dependency-free warmup matmuls: bridge the ~10us of input-DMA
            # wait at kernel start AND hold the PE-HAM activity window so
            # the first real matmuls run at K=8/8 instead of half clock.
            with tc.tile_pool(name="pwarm", bufs=1, space="PSUM") as pwarm:
                wps = pwarm.tile([128, 128], F32, name="wps")
                for i in range(100):
                    nc.tensor.matmul(
                        wps, ident, ident, start=(i == 0), stop=(i == 99)
                    )

            def rope(dst, cos_ap, sin_ap, width, tmp, eng):
                # eng: DMA queue for the rotate-half copies. Phase A1 must
                # NOT use gpsimd — its engine stream blocks in the
                # collective's wait_ge and would stall A1's DVE chain.
                t = tmp[:, :width]
                eng.dma_start(out=t[0:64, :], in_=dst[64:128, :])
                eng.dma_start(out=t[64:128, :], in_=dst[0:64, :])
                nc.vector.tensor_mul(t, t, sin_ap)
                nc.vector.tensor_mul(dst, dst, cos_ap)
                nc.vector.tensor_add(dst, dst, t)

            # ---- Phase A0: K/V projection for OWN 512 positions, K-RoPE
            # fused; then AllGather across the 4-core batch group.
            # ---- Phase A1: Q projection + fused Q-RoPE (AllGather hides
            # under this).
            # One pool scope for both phases: separate scopes would make
            # A1's xq/wq prefetch wait for A0's SBUF to release (measured
            # ~38us of PE stall at the seam).
            with (
                tc.tile_pool(name="ropetab", bufs=1) as ropetab,
                tc.tile_pool(name="ropep", bufs=2) as ropep,
                tc.tile_pool(name="xtp", bufs=4) as xtp,
                tc.tile_pool(name="wkvp", bufs=6) as wkvp,
                tc.tile_pool(name="kvchunk", bufs=1) as kvchunk,
                tc.tile_pool(name="xqp", bufs=1) as xqp,
                tc.tile_pool(name="wqp", bufs=6) as wqp,
                tc.tile_pool(name="qtab", bufs=1) as qtab,
                tc.tile_pool(name="qrtmp", bufs=2) as qrtmp,
                tc.tile_pool(name="ps_kv", bufs=2, space="PSUM") as ps_kv,
                tc.tile_pool(name="ps_tr", bufs=2, space="PSUM") as ps_tr,
                tc.tile_pool(name="ps_a", bufs=3, space="PSUM") as ps_a,
            ):
                # DMA issue order: A1-front's inputs lead BOTH queues
                # (heads 0-13 run first); A0's inputs follow (not needed
                # until ~95us). wq alternates queues so neither stream
                # gates the 6.5us/head pace.
                wkv_tiles = {}

                def load_wkv(ct):
                    for hf in range(2):
                        wblk = wkvp.tile([128, DKT // 2, 128], BF16, name="wkv")
                        nc.sync.dma_start(out=wblk, in_=wkv_d[ct, hf])
                        wkv_tiles[(ct, hf)] = wblk

                wq_tiles = {}

                def load_wq(ct, eng=None):
                    wblk = wqp.tile([128, DKT, 128], BF16, name="wq")
                    (eng or nc.scalar).dma_start(out=wblk, in_=wq_d[ct])
                    wq_tiles[ct] = wblk

                cosq = qtab.tile([128, QW], BF16, name="cosq")
                sinq = qtab.tile([128, QW], BF16, name="sinq")
                nc.scalar.dma_start(out=cosq, in_=cosq_d)
                nc.scalar.dma_start(out=sinq, in_=sinq_d)
                xq = xqp.tile([128, DKT, QW], BF16, name="xq")
                load_wq(0, nc.sync)
                nc.scalar.dma_start(out=xq[:, 0 : DKT // 4, :], in_=xq_d[0])
                load_wq(2, nc.sync)
                load_wq(1, nc.scalar)
                for ch in range(1, 4):
                    nc.scalar.dma_start(
                        out=xq[:, ch * (DKT // 4) : (ch + 1) * (DKT // 4), :],
                        in_=xq_d[ch],
                    )
                load_wq(4, nc.sync)
                load_wq(3, nc.scalar)
                load_wq(5, nc.scalar)
                # A0's inputs queue up behind (consumed from ~95us on)
                cosk = ropetab.tile([128, SC], BF16, name="cosk")
                sink = ropetab.tile([128, SC], BF16, name="sink")
                nc.scalar.dma_start(out=cosk, in_=cosk_d)
                nc.scalar.dma_start(out=sink, in_=sink_d)
                load_wkv(0)
                xts = []
                for ch in range(4):
                    xtile = xtp.tile([128, DKT // 4, SC], BF16, name="xt")
                    eng = nc.sync if ch < 2 else nc.scalar
                    eng.dma_start(out=xtile, in_=xt_d[ch])
                    for j in range(DKT // 4):
                        xts.append(xtile[:, j, :])
                for ct in range(1, 3):
                    load_wkv(ct)
                # mask table early (contiguous, host pre-transposed): B's
                # first mask-matmul must not wait on the blocked gpsimd queue
                nc.scalar.dma_start(out=mask_t, in_=mask_d)

                def a1_heads(h_lo, h_hi):
                    # Q projection + fused Q-RoPE for heads [h_lo, h_hi)
                    for ct in range(h_lo, h_hi):
                        if ct + 6 < H:
                            load_wq(ct + 6)
                        wblk = wq_tiles.pop(ct)
                        psum = ps_a.tile([128, QW], F32, name="pp")
                        for kti in range(DKT):
                            nc.tensor.matmul(
                                psum,
                                wblk[:, kti, :],
                                xq[:, kti, :],
                                start=(kti == 0),
                                stop=(kti == DKT - 1),
                            )
                        nc.vector.tensor_copy(qt[:, ct, :], psum)
                        tmp = qrtmp.tile([128, QW], BF16, name="qrtmp")
                        rope(qt[:, ct, :], cosq, sinq, QW, tmp, nc.scalar)

                # A1 front half FIRST: the uncontended early DMA window
                # feeds xq + the wq ring; A0 + the AllGather run in the
                # middle so the collective's SDMA traffic overlaps A1's
                # back half, whose weights are already buffered (wqp=6).
                a1_heads(0, 14)

                kt_own = kvchunk.tile([128, KVH, SC], BF16, name="kt_own")
                vt_own = kvchunk.tile([128, KVH, SC], BF16, name="vt_own")
                vn_own = kvchunk.tile([128, KVH, NST, 128], BF16, name="vn_own")
                for ct in range(2 * KVH):  # 0-3: K heads, 4-7: V
                    if ct + 3 < 2 * KVH:
                        load_wkv(ct + 3)
                    psum = ps_kv.tile([128, SC], F32, name="pp")
                    for hf in range(2):
                        wblk = wkv_tiles.pop((ct, hf))
                        for kti in range(DKT // 2):
                            gkt = hf * (DKT // 2) + kti
                            nc.tensor.matmul(
                                psum,
                                wblk[:, kti, :],
                                xts[gkt],
                                start=(gkt == 0),
                                stop=(gkt == DKT - 1),
                            )
                    if ct < KVH:
                        nc.vector.tensor_copy(kt_own[:, ct, :], psum)
                        tmp = ropep.tile([128, SC], BF16, name="ropetmp")
                        rope(kt_own[:, ct, :], cosk, sink, SC, tmp, nc.gpsimd)
                    else:
                        nc.vector.tensor_copy(vt_own[:, ct - KVH, :], psum)
                # V^T -> V natural (4 s-tiles x 4 heads)
                for kv in range(KVH):
                    for sti in range(NST):
                        ptr = ps_tr.tile([128, 128], BF16, name="ptr")
                        nc.tensor.transpose(
                            ptr,
                            vt_own[:, kv, sti * 128 : (sti + 1) * 128],
                            ident,
                        )
                        nc.vector.tensor_copy(vn_own[:, kv, sti, :], ptr)
                # own chunks -> DRAM bounce, AllGather, gather back
                nc.gpsimd.dma_start(
                    out=cc_in[0].rearrange("p (kv s) -> p kv s", kv=KVH),
                    in_=kt_own,
                )
                nc.gpsimd.dma_start(
                    out=cc_in[1].rearrange(
                        "p (kv st d) -> p kv st d", kv=KVH, st=NST
                    ),
                    in_=vn_own,
                )
                nc.gpsimd.collective_compute(
                    "AllGather",
                    mybir.AluOpType.bypass,
                    replica_groups=RG,
                    ins=[cc_in.ap()],
                    outs=[cc_out.ap()],
                )
                # gather-back on gpsimd: it already blocks in the
                # collective's wait_ge, and using sync here would
                # head-of-line-block phase A1's weight loads behind the
                # collective (measured: ~50us PE stall).
                for rr in range(4):
                    nc.gpsimd.dma_start(
                        out=kt_t[:, :, rr * SC : (rr + 1) * SC],
                        in_=cc_out[rr, 0].rearrange("p (kv s) -> p kv s", kv=KVH),
                    )
                    nc.gpsimd.dma_start(
                        out=vn[:, :, rr * NST : (rr + 1) * NST, :],
                        in_=cc_out[rr, 1].rearrange(
                            "p (kv st d) -> p kv st d", kv=KVH, st=NST
                        ),
                    )

                # ---- A1 back half (collective in flight underneath) ----
                a1_heads(14, H)

            # ---- Phase B+C share the wop pool so Wo prefetches during B ----
            with (
                tc.tile_pool(name="wop", bufs=8) as wop,
                tc.tile_pool(name="outp", bufs=3) as outp,
            ):
                wo_tiles = {}

                def load_wo(dc, eng):
                    # 4 ring-throttled chunk DMAs per dc. dc 0-1 go via
                    # gpsimd, whose stream resumes only after the collective
                    # wait_ge — keeping Wo's 7.4MB out of the contended
                    # collective window (the sync sequencer would race ahead
                    # and issue them mid-collective).
                    tl = []
                    for ch in range(4):
                        wt = wop.tile([128, DKT // 4, 512], BF16, name="wo")
                        eng.dma_start(out=wt, in_=wo_d[dc, ch])
                        tl.append(wt)
                    wo_tiles[dc] = tl

                load_wo(0, nc.gpsimd)
                load_wo(1, nc.gpsimd)

                # ---- Phase B: attention, 28 heads on the core's 512 q ----
                # k-tiles are packed into 6 PSUM "segments" per head (each a
                # 2-bank [128, 1024] f32 tile) so exp runs as 6 wide
                # ACTIVATEs instead of 16 narrow ones — the scalar engine's
                # ~200ns/op overhead made exp the phase bottleneck (7.5us
                # -> ~5.5us per head). The 0/1 causal-mask multiplies run on
                # GpSimd (otherwise idle); denominator accumulation stays on
                # DVE.  SEGS: list of ([(kti, column offset)...], exp width).
                # A matmul output must NOT cross a 2KB PSUM bank boundary,
                # so the 384-wide pairs sit at offsets 0/512 and exp spans
                # the (unread) garbage hole in between.
                SEGS = [
                    ([(0, 0), (1, 512)], 1024),
                    ([(2, 0), (3, 512)], 1024),
                    ([(4, 0), (5, 512)], 896),
                    ([(6, 0), (7, 512)], 896),
                    ([(8, 0), (9, 256), (10, 512), (11, 768)], 1024),
                    ([(12, 0), (13, 128), (14, 256), (15, 384)], 512),
                ]
                with (
                    tc.tile_pool(name="ptp", bufs=3) as ptp,
                    tc.tile_pool(name="accp", bufs=2) as accp,
                    tc.tile_pool(name="smallp", bufs=4) as smallp,
                    tc.tile_pool(name="ps_s", bufs=2, space="PSUM") as ps_s,
                    tc.tile_pool(name="ps_o", bufs=2, space="PSUM") as ps_o,
                    tc.tile_pool(name="ps_r", bufs=1, space="PSUM") as ps_r,
                ):
                    # deferred softmax-denominator pipeline: emit head h-1's
                    # partition-reduce / reciprocal / broadcast / normalize
                    # interleaved into head h's segment loop so the PE FIFO
                    # never stalls on the DVE accumulation chain.
                    pend_norm = []

                    def norm_step(step):
                        if not pend_norm:
                            return
                        if step == 0:
                            ph, pacc, ppo = pend_norm[0]
                            # denominator: single partition-reduce matmul
                            psum_r = ps_r.tile([1, QW], F32, name="psr")
                            nc.tensor.matmul(
                                psum_r, ones, pacc, start=True, stop=True
                            )
                            rec = smallp.tile([1, QW], F32, name="rec")
                            nc.vector.reciprocal_approx_fast(rec, psum_r)
                            rec_bf = smallp.tile([1, QW], BF16, name="rec_bf")
                            nc.vector.tensor_copy(rec_bf, rec)
                            pend_norm[0] = (ph, pacc, ppo, rec_bf)
                        elif step == 1:
                            ph, pacc, ppo, rec_bf = pend_norm[0]
                            # broadcast along partitions via PE outer product
                            # (DMA cannot do stride-0 partition reads)
                            bcast = ps_r.tile([128, QW], F32, name="bcast")
                            nc.tensor.matmul(
                                bcast, ones_row, rec_bf, start=True, stop=True
                            )
                            bcast_sb = smallp.tile([128, QW], F32, name="bcast_sb")
                            nc.vector.tensor_copy(bcast_sb, bcast)
                            pend_norm[0] = (ph, pacc, ppo, bcast_sb)
                        else:
                            ph, pacc, ppo, bcast_sb = pend_norm.pop(0)
                            # fused normalize + PSUM->SBUF, overwriting head
                            # ph's spent q columns
                            nc.vector.tensor_mul(qt[:, ph, :], ppo, bcast_sb)

                    for h in range(H):
                        kv = h // GQ
                        psum_o = ps_o.tile([128, QW], F32, name="po")
                        acc = accp.tile([128, QW], BF16, name="acc")
                        pend = []

                        def emit_pv(ent, psum_o=psum_o, kv=kv):
                            kti, lo, w, pt, off = ent
                            nc.tensor.matmul(
                                psum_o[:, lo : lo + w],
                                vn[:, kv, kti, :],
                                pt[:, off : off + w],
                                start=(kti == 0),
                                stop=(kti == NKT - 1),
                            )

                        for si, (ktis, segw) in enumerate(SEGS):
                            w = _wof(ktis[0][0])
                            lo = QW - w
                            pseg = ps_s.tile([128, 1024], F32, name="pss")
                            for kti, off in ktis:
                                nc.tensor.matmul(
                                    pseg[:, off : off + w],
                                    kt_t[:, kv, kti * 128 : (kti + 1) * 128],
                                    qt[:, h, lo:],
                                    start=True,
                                    stop=False,
                                )
                                # additive causal mask folded into the score
                                # accumulation group on the PE itself: the
                                # first live 128-block (the only one that can
                                # be diagonal or dead) gets += maskT^T @ I.
                                # Cheapest engine per op (~107ns, LDW-bound),
                                # and the extra PE density keeps HAM at
                                # K=8/8 — off-PE masking measurably dropped
                                # the PE clock to ~1.4GHz in phase B.
                                nc.tensor.matmul(
                                    pseg[:, off : off + 128],
                                    mask_t[:, kti, :],
                                    ident,
                                    start=False,
                                    stop=True,
                                )
                            pt = ptp.tile([128, 1024], BF16, name="pt")
                            nc.scalar.activation(
                                pt[:, :segw],
                                pseg[:, :segw],
                                mybir.ActivationFunctionType.Exp,
                            )
                            for kti, off in ktis:
                                # running softmax-denominator partial on DVE
                                if kti == 0:
                                    nc.vector.tensor_copy(
                                        acc, pt[:, off : off + w]
                                    )
                                else:
                                    nc.vector.tensor_add(
                                        acc[:, lo:],
                                        acc[:, lo:],
                                        pt[:, off : off + w],
                                    )
                                pend.append((kti, lo, w, pt, off))
                            # one-segment lag: PVs of segment si-1 emit here
                            while len(pend) > len(ktis):
                                emit_pv(pend.pop(0))
                            if si == 1:
                                norm_step(0)
                            elif si == 2:
                                norm_step(1)
                            elif si == 3:
                                norm_step(2)
                        while pend:
                            emit_pv(pend.pop(0))
                        pend_norm.append((h, acc, psum_o))
                    # drain the last head's normalization
                    norm_step(0)
                    norm_step(1)
                    norm_step(2)

                # ---- Phase C: o_proj (full Wo); rows are core-owned ----
                with tc.tile_pool(name="ps_c", bufs=2, space="PSUM") as ps_c:
                    for dc in range(NDC):
                        if dc + 2 < NDC:
                            load_wo(dc + 2, nc.sync)
                        tl = wo_tiles.pop(dc)
                        for q in range(NQT):
                            psum = ps_c.tile([128, 512], F32, name="pp")
                            for ct in range(DKT):
                                nc.tensor.matmul(
                                    psum,
                                    qt[:, ct, q * 128 : (q + 1) * 128],
                                    tl[ct // 7][:, ct % 7, :],
                                    start=(ct == 0),
                                    stop=(ct == DKT - 1),
                                )
                            ob = outp.tile([128, 512], BF16, name="ob")
                            nc.vector.tensor_copy(ob, psum)
                            nc.scalar.dma_start(
                                out=out_d[q, :, dc * 512 : (dc + 1) * 512],
                                in_=ob,
                            )

    nc.finalize()
    _NC_CACHE[key] = nc
    return nc


def _host_inputs(hidden_states, Wq, Wk, Wv, Wo):
    hidden = np.asarray(hidden_states, dtype=np.float32)
    Wq = np.asarray(Wq, dtype=np.float32) * np.float32(SCALE)
    Wk = np.asarray(Wk, dtype=np.float32)
    Wv = np.asarray(Wv, dtype=np.float32)
    Wo = np.asarray(Wo, dtype=np.float32)

    inv_freq = 1.0 / ROPE_THETA ** (np.arange(0, HD, 2, dtype=np.float32) / HD)
    t = np.arange(S, dtype=np.float32)
    freqs = np.outer(t, inv_freq)  # [S, 64]
    cos_t = np.cos(freqs.T)  # [64, S]
    sin_t = np.sin(freqs.T)
    cosk = np.concatenate([cos_t, cos_t], axis=0).astype(bfloat16)  # [128, S]
    sink = np.concatenate([-sin_t, sin_t], axis=0).astype(bfloat16)

    # shared weight layouts (identical for every core)
    wq = np.ascontiguousarray(
        Wq.reshape(DKT, 128, H, 128).transpose(2, 1, 0, 3)
    ).astype(bfloat16)  # [h, p, kt, c]
    wk4 = Wk.reshape(DKT, 128, KVH, 128)
    wv4 = Wv.reshape(DKT, 128, KVH, 128)
    wkv = np.empty((2 * KVH, 2, 128, DKT // 2, 128), np.float32)
    for ct in range(KVH):
        for hf in range(2):
            ktsl = slice(hf * (DKT // 2), (hf + 1) * (DKT // 2))
            wkv[ct, hf] = wk4[ktsl, :, ct, :].transpose(1, 0, 2)
            wkv[KVH + ct, hf] = wv4[ktsl, :, ct, :].transpose(1, 0, 2)
    wkv = wkv.astype(bfloat16)
    wo = np.ascontiguousarray(
        Wo.reshape(4, DKT // 4, 128, NDC, 512).transpose(3, 0, 2, 1, 4)
    ).astype(bfloat16)  # [dc, ch, p, kt, d]

    in_maps = []
    for core in range(8):
        b, r = core // 4, core % 4
        tiles = _qtiles(r)
        qpos = np.concatenate(
            [np.arange(t0 * 128, (t0 + 1) * 128) for t0 in tiles]
        )  # [512] ascending global q positions
        xq = np.ascontiguousarray(
            hidden[b][qpos].reshape(QW, 4, DKT // 4, 128).transpose(1, 3, 2, 0)
        ).astype(bfloat16)  # [ch, p, kt, q]
        # own contiguous K/V chunk: positions [r*SC, (r+1)*SC)
        xt = np.ascontiguousarray(
            hidden[b][r * SC : (r + 1) * SC]
            .reshape(SC, 4, DKT // 4, 128)
            .transpose(1, 3, 2, 0)
        ).astype(bfloat16)  # [ch, p, kt, s]
        cosq = np.ascontiguousarray(cosk[:, qpos])
        sinq = np.ascontiguousarray(sink[:, qpos])
        cosk_own = np.ascontiguousarray(cosk[:, r * SC : (r + 1) * SC])
        sink_own = np.ascontiguousarray(sink[:, r * SC : (r + 1) * SC])
        # mask[kt]: [128, 128] TRANSPOSED additive mask ([q, k] layout — it is
        # the stationary operand of a += maskT^T @ I accumulate on the PE) for
        # the FIRST live block of the suffix (columns QW-w .. QW-w+128).
        # Triangular when that block's q-tile equals kt (the diagonal),
        # all -inf when the block is non-causal (dead), all-zero otherwise.
        mask = np.zeros((NKT, 128, 128), np.float32)
        for kti in range(NKT):
            lo = QW - _wof(kti)
            kk = kti * 128 + np.arange(128)[None, :]
            qq = qpos[lo : lo + 128, None]
            mask[kti] = np.where(kk <= qq, 0.0, -30000.0)
        # pre-transposed to the on-chip [q-part, kt, k] layout so the DMA is
        # one contiguous 4KB-line stream instead of a 256B-line gather
        mask = np.ascontiguousarray(mask.transpose(1, 0, 2)).astype(bfloat16)
        in_maps.append(
            {
                "xq": xq,
                "xt": xt,
                "wq": wq,
                "wkv": wkv,
                "wo": wo,
                "cosq": cosq,
                "sinq": sinq,
                "cosk": cosk_own,
                "sink": sink_own,
                "mask": mask,
            }
        )
    return in_maps


def kernel(hidden_states, Wq, Wk, Wv, Wo, trace=False):
    nc = _build_nc()
    in_maps = _host_inputs(hidden_states, Wq, Wk, Wv, Wo)
    res = run_bass_kernel_spmd(nc, in_maps, list(range(8)), trace=trace)
    out = np.empty((B, S, D), dtype=np.float32)
    for core in range(8):
        b, r = core // 4, core % 4
        o = np.asarray(res.results[core]["out"], dtype=np.float32)
        for j, t0 in enumerate(_qtiles(r)):
            out[b, t0 * 128 : (t0 + 1) * 128, :] = o[j]
    if trace:
        kernel.last_exec_time_ns = res.exec_time_ns
    return out


# revision 37
# speedup vs baseline: 1.0639x; 1.0129x over previous
"""DreamAttention (GQA + RoPE + causal) on 8 trn2 NeuronCores.

Sharding: DP=2 over batch x sequence-parallel over q-tiles. Core c ->
(batch b = c // 4, seq rank r = c % 4). Core r owns q-tiles
[r, 7-r, 8+r, 15-r] (128 rows each, ascending) — every core gets exactly 34
k-tile-blocks of causal attention work, so the load is perfectly balanced.

K/V projection is seq-sharded: each core computes K^T/V for ONLY its own
512 contiguous positions (1/4 of S), then an AllGather over the 4-core
batch group assembles the full K/V. The collective runs on TOPSP+SDMA
silicon; it is sandwiched between the two halves of the Q projection so
its SDMA traffic overlaps PE work whose weights are already buffered
(wqp ring depth 6). This removes the 4x redundant K/V compute (~150us of
PE time) a collective-free version would pay.

All matmul operands are bf16 (fp32 PSUM accumulation). The first ~230us
is simultaneously PE- and DMA-bound (~51MB of weights/activations at
~240GB/s effective), so DMA queue ORDER is tuned: sync carries A0's
weights then Wo (ring-throttled into phase C), scalar carries xt-half +
xq + the wq ring, gpsimd carries the collective + gather-back. Host-side
layouts give every big stream >=7KB contiguous per-partition lines.

Per-core dataflow:
  - A1 heads 0-13: Q projection + fused Q-RoPE -> qt[:, h, :].
  - A0: K/V projection for own 512 positions (K-RoPE fused), V
    PE-transposed to natural layout; chunks to internal DRAM; AllGather
    [[0-3],[4-7]]; gather back into kt_t [128, 4kv, S] / vn.
  - A1 heads 14-27 (collective in flight underneath).
  - B: attention per (head, seg): k-tiles are packed into 6 two-bank
    [128,1024] PSUM segments so exp runs as 6 wide ACTIVATEs instead of
    16 narrow ones (the ACT engine costs ~200ns/op + w/1.2GHz; this cut
    exp from 7.5 to ~5.5us/head). Matmul outputs never cross a 2KB PSUM
    bank boundary (the 384-wide pairs sit at offsets 0/512). The additive
    causal mask for the first live 128-block of each k-tile rides the PE
    as a += maskT^T @ I accumulate — keeping it on-PE keeps the PE dense
    enough that HAM stays at K=8/8 (off-PE masking measurably dropped the
    PE clock). P^T -> exp -> PV with a one-segment software-pipeline lag;
    the softmax denominator (ones-matmul partition reduce + PE
    outer-product broadcast + normalize) for head h is emitted INSIDE
    head h+1's segment loop so the PE FIFO never stalls on the DVE chain.
  - C: o_proj (full Wo); attnT stationary, Wo moving, accumulate over 28
    head-chunks; output rows are core-owned -> DMA straight out as bf16.
Host reassembles the 8 cores' row-slices into the full [2, 2048, 3584] output.
"""

import math

import numpy as np
from ml_dtypes import bfloat16

import concourse.bass as bass
import concourse.mybir as mybir
import concourse.tile as tile
from concourse import bacc
from concourse.bass_utils import run_bass_kernel_spmd
from concourse.masks import make_identity

F32 = mybir.dt.float32
BF16 = mybir.dt.bfloat16

B, S, D = 2, 2048, 3584
H, KVH, HD = 28, 4, 128
ROPE_THETA = 1000000.0
GQ = H // KVH   # 7 q heads per kv head
DKT = D // 128  # 28 k-tiles over D
SC = 512        # per-core owned K/V chunk (S / 4)
NKT = S // 128  # 16 k tiles over sequence
NST = SC // 128  # 4 seq tiles per owned chunk
NDC = 7         # output D chunks of 512
NQT = 4         # q-tiles owned per core
QW = NQT * 128  # 512 q columns per core
SCALE = 1.0 / math.sqrt(HD)
PVDEPTH = 3     # attention software-pipeline depth (S runs ahead of PV)
RG = [[0, 1, 2, 3], [4, 5, 6, 7]]  # batch groups for the K/V AllGather


def _qtiles(r):
    """Ascending q-tile ids owned by seq-rank r; sum of (t+1) == 34 for all r."""
    return [r, 7 - r, 8 + r, 15 - r]


def _wof(kti):
    # Live-suffix width for k-tile kti. Rank-independent: every rank's
    # ascending tile list [t0<t1<t2<t3] satisfies t0<=3, 4<=t1<=7, 8<=t2<=11,
    # 12<=t3<=15, so #(tiles >= kti) == 4 - kti//4 for all ranks.
    return 128 * (4 - kti // 4)


_NC_CACHE = {}


def _build_nc():
    key = "nc"
    if key in _NC_CACHE:
        return _NC_CACHE[key]

    nc = bacc.Bacc("TRN2", target_bir_lowering=False, debug=False, num_devices=8)

    xq_d = nc.dram_tensor("xq", [4, 128, DKT // 4, QW], BF16, kind="ExternalInput").ap()
    xt_d = nc.dram_tensor("xt", [4, 128, DKT // 4, SC], BF16, kind="ExternalInput").ap()
    wq_d = nc.dram_tensor("wq", [H, 128, DKT, 128], BF16, kind="ExternalInput").ap()
    wkv_d = nc.dram_tensor(
        "wkv", [2 * KVH, 2, 128, DKT // 2, 128], BF16, kind="ExternalInput"
    ).ap()
    wo_d = nc.dram_tensor(
        "wo", [NDC, 4, 128, DKT // 4, 512], BF16, kind="ExternalInput"
    ).ap()
    cosq_d = nc.dram_tensor("cosq", [128, QW], BF16, kind="ExternalInput").ap()
    sinq_d = nc.dram_tensor("sinq", [128, QW], BF16, kind="ExternalInput").ap()
    cosk_d = nc.dram_tensor("cosk", [128, SC], BF16, kind="ExternalInput").ap()
    sink_d = nc.dram_tensor("sink", [128, SC], BF16, kind="ExternalInput").ap()
    mask_d = nc.dram_tensor("mask", [128, NKT, 128], BF16, kind="ExternalInput").ap()
    out_d = nc.dram_tensor("out", [NQT, 128, D], BF16, kind="ExternalOutput").ap()

    # K/V AllGather bounce buffers (collectives can't touch kernel I/O).
    # cc_in[0] = own K^T chunk [128, 4kv, 512pos]; cc_in[1] = own V natural
    # chunk [128pos-in-tile, 4kv, 4tile, 128d]. AllGather concatenates rank
    # shards along the leading axis of cc_out.
    cc_in = nc.dram_tensor("cc_in", [2, 128, 2048], BF16)
    # Shared addr_space needs >4-core groups; Local works for 4-core AG.
    cc_out = nc.dram_tensor("cc_out", [4, 2, 128, 2048], BF16)

    with tile.TileContext(nc) as tc:
        with tc.tile_pool(name="persist", bufs=1) as persist:
            # qt doubles as the attention-output buffer: att(h) overwrites
            # qt[:, h, :] once head h's scores are done.
            qt = persist.tile([128, H, QW], BF16, name="qt")
            kt_t = persist.tile([128, KVH, S], BF16, name="kt")
            vn = persist.tile([128, KVH, NKT, 128], BF16, name="vn")
            ident = persist.tile([128, 128], BF16, name="ident")
            mask_t = persist.tile([128, NKT, 128], BF16, name="mask_t")
            ones = persist.tile([128, 1], BF16, name="ones")
            ones_row = persist.tile([1, 128], BF16, name="ones_row")

            make_identity(nc, ident)
            nc.vector.memset(ones, 1.0)
            nc.vector.memset(ones_row, 1.0)

            # dependency-free warmup matmuls: bridge the ~10us of input-DMA
            # wait at kernel start AND hold the PE-HAM activity window so
            # the first real matmuls run at K=8/8 instead of half clock.
            with tc.tile_pool(name="pwarm", bufs=1, space="PSUM") as pwarm:
                wps = pwarm.tile([128, 128], F32, name="wps")
                for i in range(100):
                    nc.tensor.matmul(
                        wps, ident, ident, start=(i == 0), stop=(i == 99)
                    )

            def rope(dst, cos_ap, sin_ap, width, tmp, eng):
                # eng: DMA queue for the rotate-half copies. Phase A1 must
                # NOT use gpsimd — its engine stream blocks in the
                # collective's wait_ge and would stall A1's DVE chain.
                t = tmp[:, :width]
                eng.dma_start(out=t[0:64, :], in_=dst[64:128, :])
                eng.dma_start(out=t[64:128, :], in_=dst[0:64, :])
                nc.vector.tensor_mul(t, t, sin_ap)
                nc.vector.tensor_mul(dst, dst, cos_ap)
                nc.vector.tensor_add(dst, dst, t)

            # ---- Phase A0: K/V projection for OWN 512 positions, K-RoPE
            # fused; then AllGather across the 4-core batch group.
            # ---- Phase A1: Q projection + fused Q-RoPE (AllGather hides
            # under this).
            # One pool scope for both phases: separate scopes would make
            # A1's xq/wq prefetch wait for A0's SBUF to release (measured
            # ~38us of PE stall at the seam).
            with (
                tc.tile_pool(name="ropetab", bufs=1) as ropetab,
                tc.tile_pool(name="ropep", bufs=2) as ropep,
                tc.tile_pool(name="xtp", bufs=4) as xtp,
                tc.tile_pool(name="wkvp", bufs=6) as wkvp,
                tc.tile_pool(name="kvchunk", bufs=1) as kvchunk,
                tc.tile_pool(name="xqp", bufs=1) as xqp,
                tc.tile_pool(name="wqp", bufs=6) as wqp,
                tc.tile_pool(name="qtab", bufs=1) as qtab,
                tc.tile_pool(name="qrtmp", bufs=2) as qrtmp,
                tc.tile_pool(name="ps_kv", bufs=2, space="PSUM") as ps_kv,
                tc.tile_pool(name="ps_tr", bufs=2, space="PSUM") as ps_tr,
                tc.tile_pool(name="ps_a", bufs=3, space="PSUM") as ps_a,
            ):
                # DMA issue order: A1-front's inputs lead BOTH queues
                # (heads 0-13 run first); A0's inputs follow (not needed
                # until ~95us). wq alternates queues so neither stream
                # gates the 6.5us/head pace.
                wkv_tiles = {}

                def load_wkv(ct):
                    for hf in range(2):
                        wblk = wkvp.tile([128, DKT // 2, 128], BF16, name="wkv")
                        nc.sync.dma_start(out=wblk, in_=wkv_d[ct, hf])
                        wkv_tiles[(ct, hf)] = wblk

                wq_tiles = {}

                def load_wq(ct, eng=None):
                    wblk = wqp.tile([128, DKT, 128], BF16, name="wq")
                    (eng or nc.scalar).dma_start(out=wblk, in_=wq_d[ct])
                    wq_tiles[ct] = wblk

                cosq = qtab.tile([128, QW], BF16, name="cosq")
                sinq = qtab.tile([128, QW], BF16, name="sinq")
                nc.scalar.dma_start(out=cosq, in_=cosq_d)
                nc.scalar.dma_start(out=sinq, in_=sinq_d)
                xq = xqp.tile([128, DKT, QW], BF16, name="xq")
                load_wq(0, nc.sync)
                nc.scalar.dma_start(out=xq[:, 0 : DKT // 4, :], in_=xq_d[0])
                nc.sync.dma_start(
                    out=xq[:, 2 * (DKT // 4) : 3 * (DKT // 4), :], in_=xq_d[2]
                )
                nc.scalar.dma_start(
                    out=xq[:, DKT // 4 : 2 * (DKT // 4), :], in_=xq_d[1]
                )
                nc.sync.dma_start(out=xq[:, 3 * (DKT // 4) :, :], in_=xq_d[3])
                load_wq(1, nc.scalar)
                load_wq(2, nc.sync)
                load_wq(3, nc.scalar)
                load_wq(4, nc.sync)
                load_wq(5, nc.scalar)
                # A0's inputs queue up behind (consumed from ~95us on)
                cosk = ropetab.tile([128, SC], BF16, name="cosk")
                sink = ropetab.tile([128, SC], BF16, name="sink")
                nc.scalar.dma_start(out=cosk, in_=cosk_d)
                nc.scalar.dma_start(out=sink, in_=sink_d)
                load_wkv(0)
                xts = []
                for ch in range(4):
                    xtile = xtp.tile([128, DKT // 4, SC], BF16, name="xt")
                    eng = nc.sync if ch < 2 else nc.scalar
                    eng.dma_start(out=xtile, in_=xt_d[ch])
                    for j in range(DKT // 4):
                        xts.append(xtile[:, j, :])
                for ct in range(1, 3):
                    load_wkv(ct)
                # mask table early (contiguous, host pre-transposed): B's
                # first mask-matmul must not wait on the blocked gpsimd queue
                nc.scalar.dma_start(out=mask_t, in_=mask_d)

                def a1_heads(h_lo, h_hi):
                    # Q projection + fused Q-RoPE for heads [h_lo, h_hi)
                    for ct in range(h_lo, h_hi):
                        if ct + 6 < H:
                            load_wq(ct + 6)
                        wblk = wq_tiles.pop(ct)
                        psum = ps_a.tile([128, QW], F32, name="pp")
                        for kti in range(DKT):
                            nc.tensor.matmul(
                                psum,
                                wblk[:, kti, :],
                                xq[:, kti, :],
                                start=(kti == 0),
                                stop=(kti == DKT - 1),
                            )
                        nc.vector.tensor_copy(qt[:, ct, :], psum)
                        tmp = qrtmp.tile([128, QW], BF16, name="qrtmp")
                        rope(qt[:, ct, :], cosq, sinq, QW, tmp, nc.scalar)

                # A1 front half FIRST: the uncontended early DMA window
                # feeds xq + the wq ring; A0 + the AllGather run in the
                # middle so the collective's SDMA traffic overlaps A1's
                # back half, whose weights are already buffered (wqp=6).
                a1_heads(0, 14)

                kt_own = kvchunk.tile([128, KVH, SC], BF16, name="kt_own")
                vt_own = kvchunk.tile([128, KVH, SC], BF16, name="vt_own")
                vn_own = kvchunk.tile([128, KVH, NST, 128], BF16, name="vn_own")
                for ct in range(2 * KVH):  # 0-3: K heads, 4-7: V
                    if ct + 3 < 2 * KVH:
                        load_wkv(ct + 3)
                    psum = ps_kv.tile([128, SC], F32, name="pp")
                    for hf in range(2):
                        wblk = wkv_tiles.pop((ct, hf))
                        for kti in range(DKT // 2):
                            gkt = hf * (DKT // 2) + kti
                            nc.tensor.matmul(
                                psum,
                                wblk[:, kti, :],
                                xts[gkt],
                                start=(gkt == 0),
                                stop=(gkt == DKT - 1),
                            )
                    if ct < KVH:
                        nc.vector.tensor_copy(kt_own[:, ct, :], psum)
                        tmp = ropep.tile([128, SC], BF16, name="ropetmp")
                        rope(kt_own[:, ct, :], cosk, sink, SC, tmp, nc.gpsimd)
                    else:
                        nc.vector.tensor_copy(vt_own[:, ct - KVH, :], psum)
                # V^T -> V natural (4 s-tiles x 4 heads)
                for kv in range(KVH):
                    for sti in range(NST):
                        ptr = ps_tr.tile([128, 128], BF16, name="ptr")
                        nc.tensor.transpose(
                            ptr,
                            vt_own[:, kv, sti * 128 : (sti + 1) * 128],
                            ident,
                        )
                        nc.vector.tensor_copy(vn_own[:, kv, sti, :], ptr)
                # own chunks -> DRAM bounce, AllGather, gather back
                nc.gpsimd.dma_start(
                    out=cc_in[0].rearrange("p (kv s) -> p kv s", kv=KVH),
                    in_=kt_own,
                )
                nc.gpsimd.dma_start(
                    out=cc_in[1].rearrange(
                        "p (kv st d) -> p kv st d", kv=KVH, st=NST
                    ),
                    in_=vn_own,
                )
                nc.gpsimd.collective_compute(
                    "AllGather",
                    mybir.AluOpType.bypass,
                    replica_groups=RG,
                    ins=[cc_in.ap()],
                    outs=[cc_out.ap()],
                )
                # gather-back on gpsimd: it already blocks in the
                # collective's wait_ge, and using sync here would
                # head-of-line-block phase A1's weight loads behind the
                # collective (measured: ~50us PE stall).
                for rr in range(4):
                    nc.gpsimd.dma_start(
                        out=kt_t[:, :, rr * SC : (rr + 1) * SC],
                        in_=cc_out[rr, 0].rearrange("p (kv s) -> p kv s", kv=KVH),
                    )
                    nc.gpsimd.dma_start(
                        out=vn[:, :, rr * NST : (rr + 1) * NST, :],
                        in_=cc_out[rr, 1].rearrange(
                            "p (kv st d) -> p kv st d", kv=KVH, st=NST
                        ),
                    )

                # ---- A1 back half (collective in flight underneath) ----
                a1_heads(14, H)

            # ---- Phase B+C share the wop pool so Wo prefetches during B ----
            with (
                tc.tile_pool(name="wop", bufs=8) as wop,
                tc.tile_pool(name="outp", bufs=3) as outp,
            ):
                wo_tiles = {}

                def load_wo(dc, eng):
                    # 4 ring-throttled chunk DMAs per dc. dc 0-1 go via
                    # gpsimd, whose stream resumes only after the collective
                    # wait_ge — keeping Wo's 7.4MB out of the contended
                    # collective window (the sync sequencer would race ahead
                    # and issue them mid-collective).
                    tl = []
                    for ch in range(4):
                        wt = wop.tile([128, DKT // 4, 512], BF16, name="wo")
                        eng.dma_start(out=wt, in_=wo_d[dc, ch])
                        tl.append(wt)
                    wo_tiles[dc] = tl

                load_wo(0, nc.gpsimd)
                load_wo(1, nc.gpsimd)

                # ---- Phase B: attention, 28 heads on the core's 512 q ----
                # k-tiles are packed into 6 PSUM "segments" per head (each a
                # 2-bank [128, 1024] f32 tile) so exp runs as 6 wide
                # ACTIVATEs instead of 16 narrow ones — the scalar engine's
                # ~200ns/op overhead made exp the phase bottleneck (7.5us
                # -> ~5.5us per head). The 0/1 causal-mask multiplies run on
                # GpSimd (otherwise idle); denominator accumulation stays on
                # DVE.  SEGS: list of ([(kti, column offset)...], exp width).
                # A matmul output must NOT cross a 2KB PSUM bank boundary,
                # so the 384-wide pairs sit at offsets 0/512 and exp spans
                # the (unread) garbage hole in between.
                SEGS = [
                    ([(0, 0), (1, 512)], 1024),
                    ([(2, 0), (3, 512)], 1024),
                    ([(4, 0), (5, 512)], 896),
                    ([(6, 0), (7, 512)], 896),
                    ([(8, 0), (9, 256), (10, 512), (11, 768)], 1024),
                    ([(12, 0), (13, 128), (14, 256), (15, 384)], 512),
                ]
                with (
                    tc.tile_pool(name="ptp", bufs=3) as ptp,
                    tc.tile_pool(name="accp", bufs=2) as accp,
                    tc.tile_pool(name="smallp", bufs=4) as smallp,
                    tc.tile_pool(name="ps_s", bufs=2, space="PSUM") as ps_s,
                    tc.tile_pool(name="ps_o", bufs=2, space="PSUM") as ps_o,
                    tc.tile_pool(name="ps_r", bufs=1, space="PSUM") as ps_r,
                ):
                    # deferred softmax-denominator pipeline: emit head h-1's
                    # partition-reduce / reciprocal / broadcast / normalize
                    # interleaved into head h's segment loop so the PE FIFO
                    # never stalls on the DVE accumulation chain.
                    pend_norm = []

                    def norm_step(step):
                        if not pend_norm:
                            return
                        if step == 0:
                            ph, pacc, ppo = pend_norm[0]
                            # denominator: single partition-reduce matmul
                            psum_r = ps_r.tile([1, QW], F32, name="psr")
                            nc.tensor.matmul(
                                psum_r, ones, pacc, start=True, stop=True
                            )
                            rec = smallp.tile([1, QW], F32, name="rec")
                            nc.vector.reciprocal_approx_fast(rec, psum_r)
                            rec_bf = smallp.tile([1, QW], BF16, name="rec_bf")
                            nc.vector.tensor_copy(rec_bf, rec)
                            pend_norm[0] = (ph, pacc, ppo, rec_bf)
                        elif step == 1:
                            ph, pacc, ppo, rec_bf = pend_norm[0]
                            # broadcast along partitions via PE outer product
                            # (DMA cannot do stride-0 partition reads)
                            bcast = ps_r.tile([128, QW], F32, name="bcast")
                            nc.tensor.matmul(
                                bcast, ones_row, rec_bf, start=True, stop=True
                            )
                            bcast_sb = smallp.tile([128, QW], F32, name="bcast_sb")
                            nc.vector.tensor_copy(bcast_sb, bcast)
                            pend_norm[0] = (ph, pacc, ppo, bcast_sb)
                        else:
                            ph, pacc, ppo, bcast_sb = pend_norm.pop(0)
                            # fused normalize + PSUM->SBUF, overwriting head
                            # ph's spent q columns
                            nc.vector.tensor_mul(qt[:, ph, :], ppo, bcast_sb)

                    for h in range(H):
                        kv = h // GQ
                        psum_o = ps_o.tile([128, QW], F32, name="po")
                        acc = accp.tile([128, QW], BF16, name="acc")
                        pend = []

                        def emit_pv(ent, psum_o=psum_o, kv=kv):
                            kti, lo, w, pt, off = ent
                            nc.tensor.matmul(
                                psum_o[:, lo : lo + w],
                                vn[:, kv, kti, :],
                                pt[:, off : off + w],
                                start=(kti == 0),
                                stop=(kti == NKT - 1),
                            )

                        for si, (ktis, segw) in enumerate(SEGS):
                            w = _wof(ktis[0][0])
                            lo = QW - w
                            pseg = ps_s.tile([128, 1024], F32, name="pss")
                            for kti, off in ktis:
                                nc.tensor.matmul(
                                    pseg[:, off : off + w],
                                    kt_t[:, kv, kti * 128 : (kti + 1) * 128],
                                    qt[:, h, lo:],
                                    start=True,
                                    stop=False,
                                )
                                # additive causal mask folded into the score
                                # accumulation group on the PE itself: the
                                # first live 128-block (the only one that can
                                # be diagonal or dead) gets += maskT^T @ I.
                                # Cheapest engine per op (~107ns, LDW-bound),
                                # and the extra PE density keeps HAM at
                                # K=8/8 — off-PE masking measurably dropped
                                # the PE clock to ~1.4GHz in phase B.
                                nc.tensor.matmul(
                                    pseg[:, off : off + 128],
                                    mask_t[:, kti, :],
                                    ident,
                                    start=False,
                                    stop=True,
                                )
                            pt = ptp.tile([128, 1024], BF16, name="pt")
                            nc.scalar.activation(
                                pt[:, :segw],
                                pseg[:, :segw],
                                mybir.ActivationFunctionType.Exp,
                            )
                            for kti, off in ktis:
                                # running softmax-denominator partial on DVE
                                if kti == 0:
                                    nc.vector.tensor_copy(
                                        acc, pt[:, off : off + w]
                                    )
                                else:
                                    nc.vector.tensor_add(
                                        acc[:, lo:],
                                        acc[:, lo:],
                                        pt[:, off : off + w],
                                    )
                                pend.append((kti, lo, w, pt, off))
                            # one-segment lag: PVs of segment si-1 emit here
                            while len(pend) > len(ktis):
                                emit_pv(pend.pop(0))
                            if si == 1:
                                norm_step(0)
                            elif si == 2:
                                norm_step(1)
                            elif si == 3:
                                norm_step(2)
                        while pend:
                            emit_pv(pend.pop(0))
                        pend_norm.append((h, acc, psum_o))
                    # drain the last head's normalization
                    norm_step(0)
                    norm_step(1)
                    norm_step(2)

                # ---- Phase C: o_proj (full Wo); rows are core-owned ----
                with tc.tile_pool(name="ps_c", bufs=2, space="PSUM") as ps_c:
                    for dc in range(NDC):
                        if dc + 2 < NDC:
                            load_wo(dc + 2, nc.sync)
                        tl = wo_tiles.pop(dc)
                        for q in range(NQT):
                            psum = ps_c.tile([128, 512], F32, name="pp")
                            for ct in range(DKT):
                                nc.tensor.matmul(
                                    psum,
                                    qt[:, ct, q * 128 : (q + 1) * 128],
                                    tl[ct // 7][:, ct % 7, :],
                                    start=(ct == 0),
                                    stop=(ct == DKT - 1),
                                )
                            ob = outp.tile([128, 512], BF16, name="ob")
                            nc.vector.tensor_copy(ob, psum)
                            nc.scalar.dma_start(
                                out=out_d[q, :, dc * 512 : (dc + 1) * 512],
                                in_=ob,
                            )

    nc.finalize()
    _NC_CACHE[key] = nc
    return nc


def _host_inputs(hidden_states, Wq, Wk, Wv, Wo):
    hidden = np.asarray(hidden_states, dtype=np.float32)
    Wq = np.asarray(Wq, dtype=np.float32) * np.float32(SCALE)
    Wk = np.asarray(Wk, dtype=np.float32)
    Wv = np.asarray(Wv, dtype=np.float32)
    Wo = np.asarray(Wo, dtype=np.float32)

    inv_freq = 1.0 / ROPE_THETA ** (np.arange(0, HD, 2, dtype=np.float32) / HD)
    t = np.arange(S, dtype=np.float32)
    freqs = np.outer(t, inv_freq)  # [S, 64]
    cos_t = np.cos(freqs.T)  # [64, S]
    sin_t = np.sin(freqs.T)
    cosk = np.concatenate([cos_t, cos_t], axis=0).astype(bfloat16)  # [128, S]
    sink = np.concatenate([-sin_t, sin_t], axis=0).astype(bfloat16)

    # shared weight layouts (identical for every core)
    wq = np.ascontiguousarray(
        Wq.reshape(DKT, 128, H, 128).transpose(2, 1, 0, 3)
    ).astype(bfloat16)  # [h, p, kt, c]
    wk4 = Wk.reshape(DKT, 128, KVH, 128)
    wv4 = Wv.reshape(DKT, 128, KVH, 128)
    wkv = np.empty((2 * KVH, 2, 128, DKT // 2, 128), np.float32)
    for ct in range(KVH):
        for hf in range(2):
            ktsl = slice(hf * (DKT // 2), (hf + 1) * (DKT // 2))
            wkv[ct, hf] = wk4[ktsl, :, ct, :].transpose(1, 0, 2)
            wkv[KVH + ct, hf] = wv4[ktsl, :, ct, :].transpose(1, 0, 2)
    wkv = wkv.astype(bfloat16)
    wo = np.ascontiguousarray(
        Wo.reshape(4, DKT // 4, 128, NDC, 512).transpose(3, 0, 2, 1, 4)
    ).astype(bfloat16)  # [dc, ch, p, kt, d]

    in_maps = []
    for core in range(8):
        b, r = core // 4, core % 4
        tiles = _qtiles(r)
        qpos = np.concatenate(
            [np.arange(t0 * 128, (t0 + 1) * 128) for t0 in tiles]
        )  # [512] ascending global q positions
        xq = np.ascontiguousarray(
            hidden[b][qpos].reshape(QW, 4, DKT // 4, 128).transpose(1, 3, 2, 0)
        ).astype(bfloat16)  # [ch, p, kt, q]
        # own contiguous K/V chunk: positions [r*SC, (r+1)*SC)
        xt = np.ascontiguousarray(
            hidden[b][r * SC : (r + 1) * SC]
            .reshape(SC, 4, DKT // 4, 128)
            .transpose(1, 3, 2, 0)
        ).astype(bfloat16)  # [ch, p, kt, s]
        cosq = np.ascontiguousarray(cosk[:, qpos])
        sinq = np.ascontiguousarray(sink[:, qpos])
        cosk_own = np.ascontiguousarray(cosk[:, r * SC : (r + 1) * SC])
        sink_own = np.ascontiguousarray(sink[:, r * SC : (r + 1) * SC])
        # mask[kt]: [128, 128] TRANSPOSED additive mask ([q, k] layout — it is
        # the stationary operand of a += maskT^T @ I accumulate on the PE) for
        # the FIRST live block of the suffix (columns QW-w .. QW-w+128).
        # Triangular when that block's q-tile equals kt (the diagonal),
        # all -inf when the block is non-causal (dead), all-zero otherwise.
        mask = np.zeros((NKT, 128, 128), np.float32)
        for kti in range(NKT):
            lo = QW - _wof(kti)
            kk = kti * 128 + np.arange(128)[None, :]
            qq = qpos[lo : lo + 128, None]
            mask[kti] = np.where(kk <= qq, 0.0, -30000.0)
        # pre-transposed to the on-chip [q-part, kt, k] layout so the DMA is
        # one contiguous 4KB-line stream instead of a 256B-line gather
        mask = np.ascontiguousarray(mask.transpose(1, 0, 2)).astype(bfloat16)
        in_maps.append(
            {
                "xq": xq,
                "xt": xt,
                "wq": wq,
                "wkv": wkv,
                "wo": wo,
                "cosq": cosq,
                "sinq": sinq,
                "cosk": cosk_own,
                "sink": sink_own,
                "mask": mask,
            }
        )
    return in_maps


def kernel(hidden_states, Wq, Wk, Wv, Wo, trace=False):
    nc = _build_nc()
    in_maps = _host_inputs(hidden_states, Wq, Wk, Wv, Wo)
    res = run_bass_kernel_spmd(nc, in_maps, list(range(8)), trace=trace)
    out = np.empty((B, S, D), dtype=np.float32)
    for core in range(8):
        b, r = core // 4, core % 4
        o = np.asarray(res.results[core]["out"], dtype=np.float32)
        for j, t0 in enumerate(_qtiles(r)):
            out[b, t0 * 128 : (t0 + 1) * 128, :] = o[j]
    if trace:
        kernel.last_exec_time_ns = res.exec_time_ns
    return out
